# revision 31
# baseline (speedup 1.0000x reference)
"""Multi-head attention (B=4, S=2048, H=16, D=64, C=1024) on 8 NeuronCores.

Sharding: core c handles batch b=c//2 and head-half half=c%2 (8 heads = 512
inner dims).  Each core computes q/k/v projections for its half of the heads,
full softmax attention over S=2048, and a partial output projection through
its 512 rows of Wo.  Host sums the two partials per batch and adds the bias.

Per-core kernel layout (all matmul operands bf16, PSUM accumulation fp32):
  xt    [C=1024, S=2048]   hidden_states[b].T          (host pre-transposed)
  wq/wk/wv [C, I=512]      per-half weight columns
  wo    [I=512, C=1024]    per-half weight rows
  qT,kT [I, S] stored as 4 SBUF tiles [128, 2048]  (head pair per tile)
  v_pad [S, 8*65]          v with a ones column per head (row-sum via matmul)
  scores^T per (pair, qi-chunk, kj-tile): [kj=128, qi=512] via row-tiled
  (K=64) matmul pairs; exp on ScalarE; p@[v|1] accumulated in PSUM over kj.

v2 schedule (default flags "bcpsum,auxacc,pburst4,dmaq"):
  - constant memsets (v ones-columns) hoisted out of the repeat loop
    (a [128,8320] memset measures ~160us on HW);
  - one DMA per input tensor, all on the SP queue; output stores on the
    GPSIMD SWDGE queue so no dma_start wait can stall the ACT queue;
  - quarter-chunk p@v bursts: the scores->exp stream runs 4 kt ahead into
    an SBUF ring, then 8 pv matmuls burst, decoupling the in-order PE
    from the per-kt PE->ACT->PE round trip;
  - softmax normalization off the PSUM critical path: DVE evacuates the
    pv accumulators to SBUF immediately (banks recycle), then
    approx-reciprocal of the ones-column row-sum, K=1 PE broadcast
    matmul into dedicated PSUM banks (never the scores ring - sharing
    it convoys the whole pipeline), and a DVE multiply;
  - scores PSUM ring is a pure 2-slot double buffer; proj/outproj
    accumulators and the broadcast share the third bank pair.
Measured ~455us +-20 per iteration vs 606us for the v1 baseline (the
spread is compile-schedule lottery from Python hash randomization; the
relative error is 2.68e-3 on every draw).
"""

import functools

import numpy as np
import ml_dtypes

S = 2048          # sequence length
C = 1024          # query dim
I = 512           # inner dims per core (8 heads x 64)
HC = 8            # heads per core
D = 64            # head dim
NCORES = 8
SCALE = D ** -0.5
CT = C // 128     # 8 c-tiles
IT = I // 128     # 4 i-tiles (head pairs)
ST = S // 128     # 16 s-tiles
NQ = S // 512     # 4 qi chunks
VW = D + 1        # 65: v plus ones column


def _build(repeat=1, phases="dma,proj,attn,outproj,bcpsum,auxacc,pburst4,dmaq", fused_exp=True):
    import contextlib

    import concourse.bacc as bacc
    import concourse.tile as tile
    from concourse import mybir

    f32 = mybir.dt.float32
    bf16 = mybir.dt.bfloat16
    fp8 = mybir.dt.float8e4
    DR = mybir.MatmulPerfMode.DoubleRow
    Exp = mybir.ActivationFunctionType.Exp

    nc = bacc.Bacc("TRN2", target_bir_lowering=False, debug=False,
                   num_devices=NCORES)

    # All inputs are host-pre-shuffled to partition-major [128, ...] layouts so
    # every DMA reads long contiguous runs per partition.
    xt_d = nc.dram_tensor("xt", [128, CT * S], bf16, kind="ExternalInput").ap()
    wq_d = nc.dram_tensor("wq", [128, CT * I], bf16, kind="ExternalInput").ap()
    wk_d = nc.dram_tensor("wk", [128, CT * I], bf16, kind="ExternalInput").ap()
    wv_d = nc.dram_tensor("wv", [128, CT * I], bf16, kind="ExternalInput").ap()
    wo_d = nc.dram_tensor("wo", [128, IT * C], bf16, kind="ExternalInput").ap()
    out_d = nc.dram_tensor("out", [S, C], bf16, kind="ExternalOutput").ap()

    with tile.TileContext(nc) as tc:
        with contextlib.ExitStack() as ctx:
            pflags = set(phases.split(","))
            bc_own = "bcpsum" in pflags
            aux_acc = "auxacc" in pflags
            const = ctx.enter_context(tc.tile_pool(name="const", bufs=1))
            work = ctx.enter_context(tc.tile_pool(name="work", bufs=4))
            outp = ctx.enter_context(tc.tile_pool(name="outp", bufs=4))
            ps_pool = ctx.enter_context(
                tc.tile_pool(name="ps", bufs=(2 if bc_own else 3), space="PSUM"))
            pv_pool = ctx.enter_context(tc.tile_pool(name="pv", bufs=2, space="PSUM"))
            bc_pool = ctx.enter_context(
                tc.tile_pool(name="bcp", bufs=1, space="PSUM")) if bc_own else None
            # proj/outproj accumulators share the bc banks (used only at chunk
            # ends) so the scores ring stays a pure double-buffer.
            acc_pool = bc_pool if (bc_own and aux_acc) else ps_pool
            acc_tag = "bc" if (bc_own and aux_acc) else "ps"

            # ---- persistent tiles -------------------------------------------
            xt_sb = const.tile([128, CT, S], bf16)
            wq_sb = const.tile([128, CT, I], bf16)
            wk_sb = const.tile([128, CT, I], bf16)
            wv_sb = const.tile([128, CT, I], bf16)
            wo_sb = const.tile([128, IT, C], bf16)
            qT_sb = const.tile([128, IT, S], bf16)
            kT_sb = const.tile([128, IT, S], bf16)
            pv8 = "pv8" in pflags
            if not pv8:
                v_sb = const.tile([128, ST, HC * VW], bf16)
            VW8 = 80
            if pv8:
                v8_sb = const.tile([128, ST // 2, 2, HC, VW8], fp8)
            oT_sb = const.tile([128, IT, S], bf16)
            ones64 = const.tile([1, D], f32)
            ones64h = const.tile([1, D], bf16)

            # ---- one-time setup (outside the repeat loop) -------------------
            # ones columns per head (softmax denominator): contiguous memset of
            # the whole tile — the v copies then overwrite the 64 data columns.
            # This memset measures ~160us on HW, so it must stay out of the
            # steady-state loop.
            if pv8:
                nc.vector.memset(v8_sb, 1.0)
            else:
                nc.vector.memset(v_sb, 1.0)
            nc.vector.memset(ones64, 1.0)
            nc.vector.memset(ones64h, 1.0)

            preproj = "preproj" in pflags
            loop_entered = []

            def enter_loop():
                if repeat > 1:
                    ctx.enter_context(tc.For_i(0, repeat, 1))
                loop_entered.append(True)

            if not preproj:
                enter_loop()

            # ---- input DMAs (one per tensor, contiguous per partition) ------
            # All on the sync (SP) queue: a dma_start's wait blocks its whole
            # issuing queue, and the ACT queue must never stall (exp stream).
            dmaq = "dmaq" in pflags
            wq_eng = nc.sync if dmaq else nc.scalar

            def input_dmas(with_wo=True):
                nc.sync.dma_start(out=xt_sb,
                                  in_=xt_d.rearrange("p (t s) -> p t s", s=S))
                wq_eng.dma_start(out=wq_sb,
                                 in_=wq_d.rearrange("p (t i) -> p t i", i=I))
                wq_eng.dma_start(out=wk_sb,
                                 in_=wk_d.rearrange("p (t i) -> p t i", i=I))
                wq_eng.dma_start(out=wv_sb,
                                 in_=wv_d.rearrange("p (t i) -> p t i", i=I))
                if with_wo:
                    wq_eng.dma_start(out=wo_sb,
                                     in_=wo_d.rearrange("p (t c) -> p t c", c=C))

            input_dmas()

            phs = set(phases.split(","))

            # ---- projections -------------------------------------------------
            if "proj" in phs:
                def proj_qk_acc(it, which, nq):
                    w_sb, o_sb = ((wq_sb, qT_sb), (wk_sb, kT_sb))[which]
                    acc = acc_pool.tile([128, 1024], f32, tag=acc_tag,
                                        name="proj_ps")
                    for ct in range(CT):
                        for h2 in range(2):
                            nc.tensor.matmul(
                                acc[:, h2 * 512:(h2 + 1) * 512],
                                lhsT=w_sb[:, ct, it * 128:(it + 1) * 128],
                                rhs=xt_sb[:, ct, nq * 1024 + h2 * 512:
                                          nq * 1024 + (h2 + 1) * 512],
                                start=(ct == 0), stop=(ct == CT - 1))
                    nc.vector.tensor_copy(
                        out=o_sb[:, it, nq * 1024:(nq + 1) * 1024], in_=acc)

                def proj_qk(it):
                    for which in range(2):
                        for nq in range(S // 1024):
                            proj_qk_acc(it, which, nq)

                v_main = (None if pv8 else v_sb.rearrange(
                    "p t (h e) -> p t h e", e=VW)[:, :, :, 0:D])

                def proj_v(st):
                    acc = acc_pool.tile([128, 1024], f32, tag=acc_tag,
                                        name="v_ps")[:, 0:512]
                    for ct in range(CT):
                        nc.tensor.matmul(
                            acc,
                            lhsT=xt_sb[:, ct, st * 128:(st + 1) * 128],
                            rhs=wv_sb[:, ct, :],
                            start=(ct == 0), stop=(ct == CT - 1))
                    if pv8:
                        nc.vector.tensor_copy(
                            out=v8_sb[:, st // 2, st % 2, :, 0:D],
                            in_=acc.rearrange("p (h d) -> p h d", d=D))
                    else:
                        nc.vector.tensor_copy(
                            out=v_main[:, st],
                            in_=acc.rearrange("p (h d) -> p h d", d=D))

            # ---- attention + interleaved output projection ------------------
            def attn_chunk(hp, nq, extra=None):
                hA, hB = 2 * hp, 2 * hp + 1
                qs = slice(nq * 512, (nq + 1) * 512)
                oA = pv_pool.tile([VW, 512], f32, tag="pv", name="oA")
                oB = pv_pool.tile([VW, 512], f32, tag="pv", name="oB")
                if "pburst4" in phs:
                    # bf16 quarter-chunk bursts: scores+exp stream ahead, pv
                    # runs as 8-matmul bursts every 4 kt — decouples the
                    # in-order PE from the per-kt ACT round trip without fp8.
                    for quarter in range(4):
                        ph = work.tile([128, 4, 1024], bf16, tag="p",
                                       bufs=(3 if "ph3" in phs else 2),
                                       name="ph")
                        for k4 in range(4):
                            kt = quarter * 4 + k4
                            if extra is not None:
                                extra(kt)
                            ks = slice(kt * 128, (kt + 1) * 128)
                            sAB = ps_pool.tile([128, 1024], f32, tag="ps",
                                               name="sAB")
                            nc.tensor.matmul(
                                sAB[:, 0:512], lhsT=kT_sb[0:64, hp, ks],
                                rhs=qT_sb[0:64, hp, qs],
                                start=True, stop=True, tile_position=(0, 0))
                            nc.tensor.matmul(
                                sAB[:, 512:1024], lhsT=kT_sb[64:128, hp, ks],
                                rhs=qT_sb[64:128, hp, qs],
                                start=True, stop=True, tile_position=(64, 0))
                            nc.scalar.activation(out=ph[:, k4, :], in_=sAB,
                                                 func=Exp, scale=SCALE)
                        for k4 in range(4):
                            kt = quarter * 4 + k4
                            nc.tensor.matmul(
                                oA, lhsT=v_sb[:, kt, hA * VW:(hA + 1) * VW],
                                rhs=ph[:, k4, 0:512],
                                start=(kt == 0), stop=(kt == ST - 1))
                            nc.tensor.matmul(
                                oB, lhsT=v_sb[:, kt, hB * VW:(hB + 1) * VW],
                                rhs=ph[:, k4, 512:1024],
                                start=(kt == 0), stop=(kt == ST - 1))
                elif pv8 and "pburst" in phs:
                    # whole-chunk p buffer: scores+exp stream fully decoupled
                    # from the pv burst, which pipelines against the NEXT
                    # chunk's exp stream at chunk granularity.
                    pc = work.tile([128, ST // 2, 2, 1024], fp8, tag="pc",
                                   bufs=2, name="pc")
                    for kt in range(ST):
                        if extra is not None:
                            extra(kt)
                        ks = slice(kt * 128, (kt + 1) * 128)
                        sAB = ps_pool.tile([128, 1024], f32, tag="ps",
                                           name="sAB")
                        nc.tensor.matmul(
                            sAB[:, 0:512], lhsT=kT_sb[0:64, hp, ks],
                            rhs=qT_sb[0:64, hp, qs],
                            start=True, stop=True, tile_position=(0, 0))
                        nc.tensor.matmul(
                            sAB[:, 512:1024], lhsT=kT_sb[64:128, hp, ks],
                            rhs=qT_sb[64:128, hp, qs],
                            start=True, stop=True, tile_position=(64, 0))
                        nc.scalar.activation(out=pc[:, kt // 2, kt % 2, :],
                                             in_=sAB, func=Exp, scale=SCALE)
                    for ktp in range(ST // 2):
                        nc.tensor.matmul(
                            oA, lhsT=v8_sb[:, ktp, :, hA, 0:VW],
                            rhs=pc[:, ktp, :, 0:512],
                            start=(ktp == 0), stop=(ktp == ST // 2 - 1),
                            perf_mode=DR)
                        nc.tensor.matmul(
                            oB, lhsT=v8_sb[:, ktp, :, hB, 0:VW],
                            rhs=pc[:, ktp, :, 512:1024],
                            start=(ktp == 0), stop=(ktp == ST // 2 - 1),
                            perf_mode=DR)
                elif pv8:
                    for ktp in range(ST // 2):
                        p2 = work.tile([128, 2, 1024], fp8, tag="p", bufs=4,
                                       name="p2")
                        for j in range(2):
                            kt = 2 * ktp + j
                            if extra is not None:
                                extra(kt)
                            ks = slice(kt * 128, (kt + 1) * 128)
                            sAB = ps_pool.tile([128, 1024], f32, tag="ps",
                                               name="sAB")
                            nc.tensor.matmul(
                                sAB[:, 0:512], lhsT=kT_sb[0:64, hp, ks],
                                rhs=qT_sb[0:64, hp, qs],
                                start=True, stop=True, tile_position=(0, 0))
                            nc.tensor.matmul(
                                sAB[:, 512:1024], lhsT=kT_sb[64:128, hp, ks],
                                rhs=qT_sb[64:128, hp, qs],
                                start=True, stop=True, tile_position=(64, 0))
                            nc.scalar.activation(out=p2[:, j, :], in_=sAB,
                                                 func=Exp, scale=SCALE)
                        nc.tensor.matmul(
                            oA, lhsT=v8_sb[:, ktp, :, hA, 0:VW],
                            rhs=p2[:, :, 0:512],
                            start=(ktp == 0), stop=(ktp == ST // 2 - 1),
                            perf_mode=DR)
                        nc.tensor.matmul(
                            oB, lhsT=v8_sb[:, ktp, :, hB, 0:VW],
                            rhs=p2[:, :, 512:1024],
                            start=(ktp == 0), stop=(ktp == ST // 2 - 1),
                            perf_mode=DR)
                else:
                  for kt in range(ST):
                    if extra is not None:
                        extra(kt)
                    ks = slice(kt * 128, (kt + 1) * 128)
                    sAB = ps_pool.tile([128, 1024], f32, tag="ps", name="sAB")
                    sA, sB = sAB[:, 0:512], sAB[:, 512:1024]
                    nc.tensor.matmul(
                        sA, lhsT=kT_sb[0:64, hp, ks], rhs=qT_sb[0:64, hp, qs],
                        start=True, stop=True, tile_position=(0, 0))
                    nc.tensor.matmul(
                        sB, lhsT=kT_sb[64:128, hp, ks], rhs=qT_sb[64:128, hp, qs],
                        start=True, stop=True, tile_position=(64, 0))
                    pAB = work.tile([128, 1024], bf16, tag="p", bufs=8, name="pAB")
                    if "act512" in phs:
                        nc.scalar.activation(out=pAB[:, 0:512], in_=sA,
                                             func=Exp, scale=SCALE)
                        nc.scalar.activation(out=pAB[:, 512:1024], in_=sB,
                                             func=Exp, scale=SCALE)
                    else:
                        nc.scalar.activation(out=pAB, in_=sAB, func=Exp, scale=SCALE)
                    pvn = 256 if "pvhalf" in phs else 512
                    nc.tensor.matmul(
                        oA[:, 0:pvn], lhsT=v_sb[:, kt, hA * VW:(hA + 1) * VW],
                        rhs=pAB[:, 0:pvn],
                        start=(kt == 0), stop=(kt == ST - 1))
                    nc.tensor.matmul(
                        oB[:, 0:pvn], lhsT=v_sb[:, kt, hB * VW:(hB + 1) * VW],
                        rhs=pAB[:, 512:512 + pvn],
                        start=(kt == 0), stop=(kt == ST - 1))
                if "nonorm" in phs:
                    return
                # Normalization, off the PSUM critical path: evacuate oA/oB to
                # SBUF immediately (frees the pv banks for the next chunk),
                # then 1/denominator (approx, 18-bit), broadcast across the 64
                # head dims via a K=1 matmul, and multiply on DVE.
                oab = work.tile([VW, 1024], bf16, tag="oab", bufs=3, name="oab")
                nc.vector.tensor_copy(out=oab[:, 0:512], in_=oA)
                nc.vector.tensor_copy(out=oab[:, 512:1024], in_=oB)
                if "oldnorm" in phs:
                    rA = work.tile([1, 512], f32, tag="recip", bufs=4, name="rA")
                    rB = work.tile([1, 512], f32, tag="recip", bufs=4, name="rB")
                    nc.vector.reciprocal(out=rA, in_=oab[D:VW, 0:512])
                    nc.vector.reciprocal(out=rB, in_=oab[D:VW, 512:1024])
                    bcA = work.tile([64, 512], f32, tag="bcs", bufs=2, name="bcA")
                    bcB = work.tile([64, 512], f32, tag="bcs", bufs=2, name="bcB")
                    nc.gpsimd.partition_broadcast(bcA, rA)
                    nc.gpsimd.partition_broadcast(bcB, rB)
                    nc.vector.tensor_mul(
                        out=oT_sb[0:64, hp, qs], in0=oab[0:D, 0:512], in1=bcA)
                    nc.vector.tensor_mul(
                        out=oT_sb[64:128, hp, qs], in0=oab[0:D, 512:1024], in1=bcB)
                    return
                # the custom recip op only works at base partition 0, so plain-
                # copy the denominator row down to partition 0 first.
                oden = work.tile([1, 1024], f32, tag="oden", bufs=2, name="oden")
                nc.vector.tensor_copy(out=oden, in_=oab[D:VW, :])
                rAB = work.tile([1, 1024], f32, tag="recip", bufs=2, name="rAB")
                nc.vector.reciprocal_approx_fast(out=rAB, in_=oden)
                rABh = work.tile([1, 1024], bf16, tag="reciph", bufs=2,
                                 name="rABh")
                nc.vector.tensor_copy(out=rABh, in_=rAB)
                if bc_own:
                    bc = bc_pool.tile([128, 1024], f32, tag="bc", name="bc")
                else:
                    bc = ps_pool.tile([128, 1024], f32, tag="ps", name="bc")
                nc.tensor.matmul(bc[0:D, 0:512], lhsT=ones64h,
                                 rhs=rABh[:, 0:512], start=True, stop=True)
                nc.tensor.matmul(bc[0:D, 512:1024], lhsT=ones64h,
                                 rhs=rABh[:, 512:1024], start=True, stop=True)
                nc.vector.tensor_mul(
                    out=oT_sb[0:64, hp, qs], in0=oab[0:D, 0:512], in1=bc[0:D, 0:512])
                nc.vector.tensor_mul(
                    out=oT_sb[64:128, hp, qs], in0=oab[0:D, 512:1024],
                    in1=bc[0:D, 512:1024])

            def outproj_tile(st):
                acc = acc_pool.tile([128, 1024], f32, tag=acc_tag,
                                    name="out_ps")
                for it in range(IT):
                    for h2 in range(2):
                        nc.tensor.matmul(
                            acc[:, h2 * 512:(h2 + 1) * 512],
                            lhsT=oT_sb[:, it, st * 128:(st + 1) * 128],
                            rhs=wo_sb[:, it, h2 * 512:(h2 + 1) * 512],
                            start=(it == 0), stop=(it == IT - 1))
                ob = outp.tile([128, 1024], bf16, tag="ob", bufs=3, name="ob")
                nc.vector.tensor_copy(out=ob, in_=acc)
                (nc.gpsimd if dmaq else nc.sync).dma_start(
                    out=out_d[st * 128:(st + 1) * 128, :], in_=ob)

            if "attn" in phs:
                # pair-major chunk order; qk projections for the next pair and
                # v tiles are woven into chunk kt-steps so PE slack under the
                # ACT-bound exp stream absorbs them.
                if "proj" in phs:
                    if "noweave" in phs:
                        if preproj:
                            enter_loop()
                        for it in range(IT):
                            proj_qk(it)
                        for st in range(ST):
                            proj_v(st)
                    elif preproj:
                        # first-iteration qk(0) runs once, outside the loop;
                        # steady-state iterations get it from the pair-3 weave.
                        proj_qk(0)
                        enter_loop()
                        input_dmas()
                        for st in range(4):
                            proj_v(st)
                    else:
                        proj_qk(0)
                        for st in range(4):
                            proj_v(st)

                def make_extra(hp, nq):
                    if "proj" not in phs or "noweave" in phs:
                        return None
                    def extra(kt):
                        if hp == 0 and nq == 0 and 4 + kt < ST:
                            proj_v(4 + kt)
                        wrap = preproj and hp == IT - 1
                        if (hp < IT - 1 or wrap) and nq in (1, 2) and kt in (3, 11):
                            acc_idx = (nq - 1) * 2 + (0 if kt == 3 else 1)
                            proj_qk_acc((hp + 1) % IT, acc_idx // 2, acc_idx % 2)
                    return extra

                for hp in range(IT):
                    for nq in range(NQ):
                        attn_chunk(hp, nq, make_extra(hp, nq))
                        if (hp == IT - 1 and "outproj" in phs
                                and "outlate" not in phs):
                            for st in range(4 * nq, 4 * (nq + 1)):
                                outproj_tile(st)
                if "outproj" in phs and "outlate" in phs:
                    for st in range(ST):
                        outproj_tile(st)
            else:
                if preproj:
                    enter_loop()
                if "proj" in phs:
                    for it in range(IT):
                        proj_qk(it)
                    for st in range(ST):
                        proj_v(st)
                if "outproj" in phs:
                    for st in range(ST):
                        outproj_tile(st)

    nc.compile()
    return nc


@functools.lru_cache(maxsize=8)
def _built(repeat=1, phases="dma,proj,attn,outproj,bcpsum,auxacc,pburst4,dmaq", fused_exp=True):
    return _build(repeat, phases, fused_exp)


def _pm(a):
    """[T*128, F] -> partition-major [128, T*F] (bf16)."""
    T = a.shape[0] // 128
    return np.ascontiguousarray(
        a.reshape(T, 128, a.shape[1]).swapaxes(0, 1).reshape(128, -1)
    ).astype(ml_dtypes.bfloat16)


def _in_maps(hidden_states, Wq, Wk, Wv, Wo):
    maps = []
    for c in range(NCORES):
        b, half = divmod(c, 2)
        sl = slice(half * I, (half + 1) * I)
        maps.append({
            "xt": _pm(np.ascontiguousarray(hidden_states[b].T)),
            "wq": _pm(Wq[:, sl]),
            "wk": _pm(Wk[:, sl]),
            "wv": _pm(Wv[:, sl]),
            "wo": _pm(Wo[sl, :]),
        })
    return maps


@functools.lru_cache(maxsize=1)
def _runner():
    """Compile the SPMD program once and return a function
    maps -> list of per-core output dicts."""
    import jax
    from jax.sharding import Mesh, PartitionSpec, NamedSharding
    from jax.experimental.shard_map import shard_map

    import concourse.mybir as mybir
    from concourse.bass2jax import (
        _bass_exec_p, install_neuronx_cc_hook, partition_id_tensor)

    nc = _built()
    install_neuronx_cc_hook()
    partition_name = nc.partition_id_tensor.name if nc.partition_id_tensor else None

    in_names, out_names, out_avals, zero_outs = [], [], [], []
    for alloc in nc.m.functions[0].allocations:
        if not isinstance(alloc, mybir.MemoryLocationSet):
            continue
        name = alloc.memorylocations[0].name
        if alloc.kind == "ExternalInput":
            if name != partition_name:
                in_names.append(name)
        elif alloc.kind == "ExternalOutput":
            out_names.append(name)
            shape = tuple(alloc.tensor_shape)
            dtype = mybir.dt.np(alloc.dtype)
            out_avals.append(jax.core.ShapedArray(shape, dtype))
            zero_outs.append(np.zeros(shape, dtype))
    n_params = len(in_names)
    all_in_names = in_names + out_names
    if partition_name is not None:
        all_in_names = all_in_names + [partition_name]

    def _body(*args):
        operands = list(args)
        if partition_name is not None:
            operands.append(partition_id_tensor())
        return tuple(_bass_exec_p.bind(
            *operands,
            out_avals=tuple(out_avals),
            in_names=tuple(all_in_names),
            out_names=tuple(out_names),
            lowering_input_output_aliases=(),
            sim_require_finite=True,
            sim_require_nnan=True,
            nc=nc,
        ))

    devices = jax.devices()[:NCORES]
    mesh = Mesh(np.asarray(devices), ("core",))
    in_specs = (PartitionSpec("core"),) * (n_params + len(out_names))
    out_specs = (PartitionSpec("core"),) * len(out_names)
    sharded = jax.jit(
        shard_map(_body, mesh=mesh, in_specs=in_specs, out_specs=out_specs,
                  check_rep=False),
        keep_unused=True,
    )
    sharding = NamedSharding(mesh, PartitionSpec("core"))
    dev_zero = [jax.device_put(
        np.zeros((NCORES * z.shape[0], *z.shape[1:]), z.dtype), sharding)
        for z in zero_outs]

    def run(maps):
        concat_in = [np.concatenate([np.asarray(maps[c][n]) for c in range(NCORES)],
                                    axis=0) for n in in_names]
        dev_in = [jax.device_put(a, sharding) for a in concat_in]
        out_arrs = sharded(*dev_in, *dev_zero)
        return [
            {n: np.asarray(out_arrs[i]).reshape(NCORES, *out_avals[i].shape)[c]
             for i, n in enumerate(out_names)}
            for c in range(NCORES)
        ]

    return run


def kernel(hidden_states, Wq, Wk, Wv, Wo, bo):
    maps = _in_maps(np.asarray(hidden_states), np.asarray(Wq), np.asarray(Wk),
                    np.asarray(Wv), np.asarray(Wo))
    results = _runner()(maps)
    B = hidden_states.shape[0]
    out = np.empty((B, S, C), np.float32)
    for b in range(B):
        out[b] = (results[2 * b]["out"].astype(np.float32)
                  + results[2 * b + 1]["out"].astype(np.float32))
    out += np.asarray(bo, np.float32)
    return out


# revision 34
# speedup vs baseline: 1.0733x; 1.0733x over previous
"""Multi-head attention (B=4, S=2048, H=16, D=64, C=1024) on 8 NeuronCores.

Sharding: core c handles batch b=c//2 and head-half half=c%2 (8 heads = 512
inner dims).  Each core computes q/k/v projections for its half of the heads,
full softmax attention over S=2048, and a partial output projection through
its 512 rows of Wo.  Host sums the two partials per batch and adds the bias.

Per-core kernel layout (all matmul operands bf16, PSUM accumulation fp32):
  xt    [C=1024, S=2048]   hidden_states[b].T          (host pre-transposed)
  wq/wk/wv [C, I=512]      per-half weight columns
  wo    [I=512, C=1024]    per-half weight rows
  qT,kT [I, S] stored as 4 SBUF tiles [128, 2048]  (head pair per tile)
  v_pad [S, 8*65]          v with a ones column per head (row-sum via matmul)
  scores^T per (pair, qi-chunk, kj-tile): [kj=128, qi=512] via row-tiled
  (K=64) matmul pairs; exp on ScalarE; p@[v|1] accumulated in PSUM over kj.

v2 schedule (default flags "bcpsum,auxacc,pburst4,dmaq"):
  - constant memsets (v ones-columns) hoisted out of the repeat loop
    (a [128,8320] memset measures ~160us on HW);
  - one DMA per input tensor, all on the SP queue; output stores on the
    GPSIMD SWDGE queue so no dma_start wait can stall the ACT queue;
  - quarter-chunk p@v bursts: the scores->exp stream runs 4 kt ahead into
    an SBUF ring, then 8 pv matmuls burst, decoupling the in-order PE
    from the per-kt PE->ACT->PE round trip;
  - softmax normalization off the PSUM critical path: DVE evacuates the
    pv accumulators to SBUF immediately (banks recycle), then
    approx-reciprocal of the ones-column row-sum, K=1 PE broadcast
    matmul into dedicated PSUM banks (never the scores ring - sharing
    it convoys the whole pipeline), and a DVE multiply;
  - scores PSUM ring is a pure 2-slot double buffer; proj/outproj
    accumulators and the broadcast share the third bank pair.
Measured ~455us +-20 per iteration vs 606us for the v1 baseline (the
spread is compile-schedule lottery from Python hash randomization; the
relative error is 2.68e-3 on every draw).
"""

import functools

import numpy as np
import ml_dtypes

S = 2048          # sequence length
C = 1024          # query dim
I = 512           # inner dims per core (8 heads x 64)
HC = 8            # heads per core
D = 64            # head dim
NCORES = 8
SCALE = D ** -0.5
CT = C // 128     # 8 c-tiles
IT = I // 128     # 4 i-tiles (head pairs)
ST = S // 128     # 16 s-tiles
NQ = S // 512     # 4 qi chunks
VW = D + 1        # 65: v plus ones column


def _build(repeat=1, phases="dma,proj,attn,outproj,bcpsum,auxacc,pburst4,dmaq,gpbc", fused_exp=True):
    import contextlib

    import concourse.bacc as bacc
    import concourse.tile as tile
    from concourse import mybir

    f32 = mybir.dt.float32
    bf16 = mybir.dt.bfloat16
    fp8 = mybir.dt.float8e4
    DR = mybir.MatmulPerfMode.DoubleRow
    Exp = mybir.ActivationFunctionType.Exp

    nc = bacc.Bacc("TRN2", target_bir_lowering=False, debug=False,
                   num_devices=NCORES)

    # All inputs are host-pre-shuffled to partition-major [128, ...] layouts so
    # every DMA reads long contiguous runs per partition.
    xt_d = nc.dram_tensor("xt", [128, CT * S], bf16, kind="ExternalInput").ap()
    wq_d = nc.dram_tensor("wq", [128, CT * I], bf16, kind="ExternalInput").ap()
    wk_d = nc.dram_tensor("wk", [128, CT * I], bf16, kind="ExternalInput").ap()
    wv_d = nc.dram_tensor("wv", [128, CT * I], bf16, kind="ExternalInput").ap()
    wo_d = nc.dram_tensor("wo", [128, IT * C], bf16, kind="ExternalInput").ap()
    out_d = nc.dram_tensor("out", [S, C], bf16, kind="ExternalOutput").ap()

    with tile.TileContext(nc) as tc:
        with contextlib.ExitStack() as ctx:
            pflags = set(phases.split(","))
            ps3 = "ps3" in pflags and "gpbc" in pflags
            bc_own = "bcpsum" in pflags and not ps3
            aux_acc = "auxacc" in pflags
            const = ctx.enter_context(tc.tile_pool(name="const", bufs=1))
            work = ctx.enter_context(tc.tile_pool(name="work", bufs=4))
            outp = ctx.enter_context(tc.tile_pool(name="outp", bufs=4))
            ps_pool = ctx.enter_context(
                tc.tile_pool(name="ps", bufs=(2 if bc_own else 3), space="PSUM"))
            # with gpbc+ps3 the broadcast lives in SBUF: scores get 3 slots
            # and the proj/outproj accumulators ride the ring's spare slot.
            pv_pool = ctx.enter_context(tc.tile_pool(name="pv", bufs=2, space="PSUM"))
            bc_pool = ctx.enter_context(
                tc.tile_pool(name="bcp", bufs=1, space="PSUM")) if bc_own else None
            # proj/outproj accumulators share the bc banks (used only at chunk
            # ends) so the scores ring stays a pure double-buffer.
            acc_pool = bc_pool if (bc_own and aux_acc) else ps_pool
            acc_tag = "bc" if (bc_own and aux_acc) else "ps"

            # ---- persistent tiles -------------------------------------------
            xt_sb = const.tile([128, CT, S], bf16)
            wq_sb = const.tile([128, CT, I], bf16)
            wk_sb = const.tile([128, CT, I], bf16)
            wv_sb = const.tile([128, CT, I], bf16)
            wo_sb = const.tile([128, IT, C], bf16)
            qT_sb = const.tile([128, IT, S], bf16)
            kT_sb = const.tile([128, IT, S], bf16)
            pv8 = "pv8" in pflags
            if not pv8:
                v_sb = const.tile([128, ST, HC * VW], bf16)
            VW8 = 80
            if pv8:
                v8_sb = const.tile([128, ST // 2, 2, HC, VW8], fp8)
            oT_sb = const.tile([128, IT, S], bf16)
            ones64 = const.tile([1, D], f32)
            ones64h = const.tile([1, D], bf16)

            # ---- one-time setup (outside the repeat loop) -------------------
            # ones columns per head (softmax denominator): contiguous memset of
            # the whole tile — the v copies then overwrite the 64 data columns.
            # This memset measures ~160us on HW, so it must stay out of the
            # steady-state loop.
            if pv8:
                nc.vector.memset(v8_sb, 1.0)
            else:
                nc.vector.memset(v_sb, 1.0)
            nc.vector.memset(ones64, 1.0)
            nc.vector.memset(ones64h, 1.0)

            preproj = "preproj" in pflags
            loop_entered = []

            def enter_loop():
                if repeat > 1:
                    ctx.enter_context(tc.For_i(0, repeat, 1))
                loop_entered.append(True)

            if not preproj:
                enter_loop()

            # ---- input DMAs (one per tensor, contiguous per partition) ------
            # All on the sync (SP) queue: a dma_start's wait blocks its whole
            # issuing queue, and the ACT queue must never stall (exp stream).
            dmaq = "dmaq" in pflags
            wq_eng = nc.sync if dmaq else nc.scalar

            def input_dmas(with_wo=True):
                nc.sync.dma_start(out=xt_sb,
                                  in_=xt_d.rearrange("p (t s) -> p t s", s=S))
                wq_eng.dma_start(out=wq_sb,
                                 in_=wq_d.rearrange("p (t i) -> p t i", i=I))
                wq_eng.dma_start(out=wk_sb,
                                 in_=wk_d.rearrange("p (t i) -> p t i", i=I))
                wq_eng.dma_start(out=wv_sb,
                                 in_=wv_d.rearrange("p (t i) -> p t i", i=I))
                if with_wo:
                    wq_eng.dma_start(out=wo_sb,
                                     in_=wo_d.rearrange("p (t c) -> p t c", c=C))

            input_dmas()

            phs = set(phases.split(","))

            # ---- projections -------------------------------------------------
            if "proj" in phs:
                def proj_qk_acc(it, which, nq):
                    w_sb, o_sb = ((wq_sb, qT_sb), (wk_sb, kT_sb))[which]
                    acc = acc_pool.tile([128, 1024], f32, tag=acc_tag,
                                        name="proj_ps")
                    for ct in range(CT):
                        for h2 in range(2):
                            nc.tensor.matmul(
                                acc[:, h2 * 512:(h2 + 1) * 512],
                                lhsT=w_sb[:, ct, it * 128:(it + 1) * 128],
                                rhs=xt_sb[:, ct, nq * 1024 + h2 * 512:
                                          nq * 1024 + (h2 + 1) * 512],
                                start=(ct == 0), stop=(ct == CT - 1))
                    nc.vector.tensor_copy(
                        out=o_sb[:, it, nq * 1024:(nq + 1) * 1024], in_=acc)

                def proj_qk(it):
                    for which in range(2):
                        for nq in range(S // 1024):
                            proj_qk_acc(it, which, nq)

                v_main = (None if pv8 else v_sb.rearrange(
                    "p t (h e) -> p t h e", e=VW)[:, :, :, 0:D])

                def proj_v(st):
                    acc = acc_pool.tile([128, 1024], f32, tag=acc_tag,
                                        name="v_ps")[:, 0:512]
                    for ct in range(CT):
                        nc.tensor.matmul(
                            acc,
                            lhsT=xt_sb[:, ct, st * 128:(st + 1) * 128],
                            rhs=wv_sb[:, ct, :],
                            start=(ct == 0), stop=(ct == CT - 1))
                    if pv8:
                        nc.vector.tensor_copy(
                            out=v8_sb[:, st // 2, st % 2, :, 0:D],
                            in_=acc.rearrange("p (h d) -> p h d", d=D))
                    else:
                        nc.vector.tensor_copy(
                            out=v_main[:, st],
                            in_=acc.rearrange("p (h d) -> p h d", d=D))

            # ---- attention + interleaved output projection ------------------
            def attn_chunk(hp, nq, extra=None):
                hA, hB = 2 * hp, 2 * hp + 1
                qs = slice(nq * 512, (nq + 1) * 512)
                oA = pv_pool.tile([VW, 512], f32, tag="pv", name="oA")
                oB = pv_pool.tile([VW, 512], f32, tag="pv", name="oB")
                if "pburst4" in phs:
                    # bf16 quarter-chunk bursts: scores+exp stream ahead, pv
                    # runs as 8-matmul bursts every 4 kt — decouples the
                    # in-order PE from the per-kt ACT round trip without fp8.
                    for quarter in range(4):
                        ph = work.tile([128, 4, 1024], bf16, tag="p",
                                       bufs=(3 if "ph3" in phs else 2),
                                       name="ph")
                        for k4 in range(4):
                            kt = quarter * 4 + k4
                            if extra is not None:
                                extra(kt)
                            ks = slice(kt * 128, (kt + 1) * 128)
                            sAB = ps_pool.tile([128, 1024], f32, tag="ps",
                                               name="sAB")
                            nc.tensor.matmul(
                                sAB[:, 0:512], lhsT=kT_sb[0:64, hp, ks],
                                rhs=qT_sb[0:64, hp, qs],
                                start=True, stop=True, tile_position=(0, 0))
                            nc.tensor.matmul(
                                sAB[:, 512:1024], lhsT=kT_sb[64:128, hp, ks],
                                rhs=qT_sb[64:128, hp, qs],
                                start=True, stop=True, tile_position=(64, 0))
                            nc.scalar.activation(out=ph[:, k4, :], in_=sAB,
                                                 func=Exp, scale=SCALE)
                        for k4 in range(4):
                            kt = quarter * 4 + k4
                            nc.tensor.matmul(
                                oA, lhsT=v_sb[:, kt, hA * VW:(hA + 1) * VW],
                                rhs=ph[:, k4, 0:512],
                                start=(kt == 0), stop=(kt == ST - 1))
                            nc.tensor.matmul(
                                oB, lhsT=v_sb[:, kt, hB * VW:(hB + 1) * VW],
                                rhs=ph[:, k4, 512:1024],
                                start=(kt == 0), stop=(kt == ST - 1))
                elif pv8 and "pburst" in phs:
                    # whole-chunk p buffer: scores+exp stream fully decoupled
                    # from the pv burst, which pipelines against the NEXT
                    # chunk's exp stream at chunk granularity.
                    pc = work.tile([128, ST // 2, 2, 1024], fp8, tag="pc",
                                   bufs=2, name="pc")
                    for kt in range(ST):
                        if extra is not None:
                            extra(kt)
                        ks = slice(kt * 128, (kt + 1) * 128)
                        sAB = ps_pool.tile([128, 1024], f32, tag="ps",
                                           name="sAB")
                        nc.tensor.matmul(
                            sAB[:, 0:512], lhsT=kT_sb[0:64, hp, ks],
                            rhs=qT_sb[0:64, hp, qs],
                            start=True, stop=True, tile_position=(0, 0))
                        nc.tensor.matmul(
                            sAB[:, 512:1024], lhsT=kT_sb[64:128, hp, ks],
                            rhs=qT_sb[64:128, hp, qs],
                            start=True, stop=True, tile_position=(64, 0))
                        nc.scalar.activation(out=pc[:, kt // 2, kt % 2, :],
                                             in_=sAB, func=Exp, scale=SCALE)
                    for ktp in range(ST // 2):
                        nc.tensor.matmul(
                            oA, lhsT=v8_sb[:, ktp, :, hA, 0:VW],
                            rhs=pc[:, ktp, :, 0:512],
                            start=(ktp == 0), stop=(ktp == ST // 2 - 1),
                            perf_mode=DR)
                        nc.tensor.matmul(
                            oB, lhsT=v8_sb[:, ktp, :, hB, 0:VW],
                            rhs=pc[:, ktp, :, 512:1024],
                            start=(ktp == 0), stop=(ktp == ST // 2 - 1),
                            perf_mode=DR)
                elif pv8:
                    for ktp in range(ST // 2):
                        p2 = work.tile([128, 2, 1024], fp8, tag="p", bufs=4,
                                       name="p2")
                        for j in range(2):
                            kt = 2 * ktp + j
                            if extra is not None:
                                extra(kt)
                            ks = slice(kt * 128, (kt + 1) * 128)
                            sAB = ps_pool.tile([128, 1024], f32, tag="ps",
                                               name="sAB")
                            nc.tensor.matmul(
                                sAB[:, 0:512], lhsT=kT_sb[0:64, hp, ks],
                                rhs=qT_sb[0:64, hp, qs],
                                start=True, stop=True, tile_position=(0, 0))
                            nc.tensor.matmul(
                                sAB[:, 512:1024], lhsT=kT_sb[64:128, hp, ks],
                                rhs=qT_sb[64:128, hp, qs],
                                start=True, stop=True, tile_position=(64, 0))
                            nc.scalar.activation(out=p2[:, j, :], in_=sAB,
                                                 func=Exp, scale=SCALE)
                        nc.tensor.matmul(
                            oA, lhsT=v8_sb[:, ktp, :, hA, 0:VW],
                            rhs=p2[:, :, 0:512],
                            start=(ktp == 0), stop=(ktp == ST // 2 - 1),
                            perf_mode=DR)
                        nc.tensor.matmul(
                            oB, lhsT=v8_sb[:, ktp, :, hB, 0:VW],
                            rhs=p2[:, :, 512:1024],
                            start=(ktp == 0), stop=(ktp == ST // 2 - 1),
                            perf_mode=DR)
                else:
                  for kt in range(ST):
                    if extra is not None:
                        extra(kt)
                    ks = slice(kt * 128, (kt + 1) * 128)
                    sAB = ps_pool.tile([128, 1024], f32, tag="ps", name="sAB")
                    sA, sB = sAB[:, 0:512], sAB[:, 512:1024]
                    nc.tensor.matmul(
                        sA, lhsT=kT_sb[0:64, hp, ks], rhs=qT_sb[0:64, hp, qs],
                        start=True, stop=True, tile_position=(0, 0))
                    nc.tensor.matmul(
                        sB, lhsT=kT_sb[64:128, hp, ks], rhs=qT_sb[64:128, hp, qs],
                        start=True, stop=True, tile_position=(64, 0))
                    pAB = work.tile([128, 1024], bf16, tag="p", bufs=8, name="pAB")
                    if "act512" in phs:
                        nc.scalar.activation(out=pAB[:, 0:512], in_=sA,
                                             func=Exp, scale=SCALE)
                        nc.scalar.activation(out=pAB[:, 512:1024], in_=sB,
                                             func=Exp, scale=SCALE)
                    else:
                        nc.scalar.activation(out=pAB, in_=sAB, func=Exp, scale=SCALE)
                    pvn = 256 if "pvhalf" in phs else 512
                    nc.tensor.matmul(
                        oA[:, 0:pvn], lhsT=v_sb[:, kt, hA * VW:(hA + 1) * VW],
                        rhs=pAB[:, 0:pvn],
                        start=(kt == 0), stop=(kt == ST - 1))
                    nc.tensor.matmul(
                        oB[:, 0:pvn], lhsT=v_sb[:, kt, hB * VW:(hB + 1) * VW],
                        rhs=pAB[:, 512:512 + pvn],
                        start=(kt == 0), stop=(kt == ST - 1))
                if "nonorm" in phs:
                    return
                # Normalization, off the PSUM critical path: evacuate oA/oB to
                # SBUF immediately (frees the pv banks for the next chunk),
                # then 1/denominator (approx, 18-bit), broadcast across the 64
                # head dims via a K=1 matmul, and multiply on DVE.
                oab = work.tile([VW, 1024], bf16, tag="oab", bufs=3, name="oab")
                nc.vector.tensor_copy(out=oab[:, 0:512], in_=oA)
                nc.vector.tensor_copy(out=oab[:, 512:1024], in_=oB)
                if "oldnorm" in phs:
                    rA = work.tile([1, 512], f32, tag="recip", bufs=4, name="rA")
                    rB = work.tile([1, 512], f32, tag="recip", bufs=4, name="rB")
                    nc.vector.reciprocal(out=rA, in_=oab[D:VW, 0:512])
                    nc.vector.reciprocal(out=rB, in_=oab[D:VW, 512:1024])
                    bcA = work.tile([64, 512], f32, tag="bcs", bufs=2, name="bcA")
                    bcB = work.tile([64, 512], f32, tag="bcs", bufs=2, name="bcB")
                    nc.gpsimd.partition_broadcast(bcA, rA)
                    nc.gpsimd.partition_broadcast(bcB, rB)
                    nc.vector.tensor_mul(
                        out=oT_sb[0:64, hp, qs], in0=oab[0:D, 0:512], in1=bcA)
                    nc.vector.tensor_mul(
                        out=oT_sb[64:128, hp, qs], in0=oab[0:D, 512:1024], in1=bcB)
                    return
                # the custom recip op only works at base partition 0, so plain-
                # copy the denominator row down to partition 0 first.
                oden = work.tile([1, 1024], f32, tag="oden", bufs=2, name="oden")
                nc.vector.tensor_copy(out=oden, in_=oab[D:VW, :])
                rAB = work.tile([1, 1024], f32, tag="recip", bufs=2, name="rAB")
                nc.vector.reciprocal_approx_fast(out=rAB, in_=oden)
                if "gpbc" in phs:
                    bcg = work.tile([D, 1024], f32, tag="bcg", bufs=2,
                                    name="bcg")
                    nc.gpsimd.partition_broadcast(bcg, rAB)
                    nc.vector.tensor_mul(
                        out=oT_sb[0:64, hp, qs], in0=oab[0:D, 0:512],
                        in1=bcg[:, 0:512])
                    nc.vector.tensor_mul(
                        out=oT_sb[64:128, hp, qs], in0=oab[0:D, 512:1024],
                        in1=bcg[:, 512:1024])
                    return
                rABh = work.tile([1, 1024], bf16, tag="reciph", bufs=2,
                                 name="rABh")
                nc.vector.tensor_copy(out=rABh, in_=rAB)
                if bc_own:
                    bc = bc_pool.tile([128, 1024], f32, tag="bc", name="bc")
                else:
                    bc = ps_pool.tile([128, 1024], f32, tag="ps", name="bc")
                nc.tensor.matmul(bc[0:D, 0:512], lhsT=ones64h,
                                 rhs=rABh[:, 0:512], start=True, stop=True)
                nc.tensor.matmul(bc[0:D, 512:1024], lhsT=ones64h,
                                 rhs=rABh[:, 512:1024], start=True, stop=True)
                nc.vector.tensor_mul(
                    out=oT_sb[0:64, hp, qs], in0=oab[0:D, 0:512], in1=bc[0:D, 0:512])
                nc.vector.tensor_mul(
                    out=oT_sb[64:128, hp, qs], in0=oab[0:D, 512:1024],
                    in1=bc[0:D, 512:1024])

            def outproj_tile(st):
                acc = acc_pool.tile([128, 1024], f32, tag=acc_tag,
                                    name="out_ps")
                for it in range(IT):
                    for h2 in range(2):
                        nc.tensor.matmul(
                            acc[:, h2 * 512:(h2 + 1) * 512],
                            lhsT=oT_sb[:, it, st * 128:(st + 1) * 128],
                            rhs=wo_sb[:, it, h2 * 512:(h2 + 1) * 512],
                            start=(it == 0), stop=(it == IT - 1))
                ob = outp.tile([128, 1024], bf16, tag="ob", bufs=3, name="ob")
                nc.vector.tensor_copy(out=ob, in_=acc)
                (nc.gpsimd if dmaq else nc.sync).dma_start(
                    out=out_d[st * 128:(st + 1) * 128, :], in_=ob)

            if "attn" in phs:
                # pair-major chunk order; qk projections for the next pair and
                # v tiles are woven into chunk kt-steps so PE slack under the
                # ACT-bound exp stream absorbs them.
                if "proj" in phs:
                    if "noweave" in phs:
                        if preproj:
                            enter_loop()
                        for it in range(IT):
                            proj_qk(it)
                        for st in range(ST):
                            proj_v(st)
                    elif preproj:
                        # first-iteration qk(0) runs once, outside the loop;
                        # steady-state iterations get it from the pair-3 weave.
                        proj_qk(0)
                        enter_loop()
                        input_dmas()
                        for st in range(4):
                            proj_v(st)
                    else:
                        proj_qk(0)
                        for st in range(4):
                            proj_v(st)

                def make_extra(hp, nq):
                    if "proj" not in phs or "noweave" in phs:
                        return None
                    def extra(kt):
                        if hp == 0 and nq == 0 and 4 + kt < ST:
                            proj_v(4 + kt)
                        wrap = preproj and hp == IT - 1
                        if (hp < IT - 1 or wrap) and nq in (1, 2) and kt in (3, 11):
                            acc_idx = (nq - 1) * 2 + (0 if kt == 3 else 1)
                            proj_qk_acc((hp + 1) % IT, acc_idx // 2, acc_idx % 2)
                    return extra

                for hp in range(IT):
                    for nq in range(NQ):
                        attn_chunk(hp, nq, make_extra(hp, nq))
                        if (hp == IT - 1 and "outproj" in phs
                                and "outlate" not in phs):
                            for st in range(4 * nq, 4 * (nq + 1)):
                                outproj_tile(st)
                if "outproj" in phs and "outlate" in phs:
                    for st in range(ST):
                        outproj_tile(st)
            else:
                if preproj:
                    enter_loop()
                if "proj" in phs:
                    for it in range(IT):
                        proj_qk(it)
                    for st in range(ST):
                        proj_v(st)
                if "outproj" in phs:
                    for st in range(ST):
                        outproj_tile(st)

    nc.compile()
    return nc


@functools.lru_cache(maxsize=8)
def _built(repeat=1, phases="dma,proj,attn,outproj,bcpsum,auxacc,pburst4,dmaq,gpbc", fused_exp=True):
    return _build(repeat, phases, fused_exp)


def _pm(a):
    """[T*128, F] -> partition-major [128, T*F] (bf16)."""
    T = a.shape[0] // 128
    return np.ascontiguousarray(
        a.reshape(T, 128, a.shape[1]).swapaxes(0, 1).reshape(128, -1)
    ).astype(ml_dtypes.bfloat16)


def _in_maps(hidden_states, Wq, Wk, Wv, Wo):
    maps = []
    for c in range(NCORES):
        b, half = divmod(c, 2)
        sl = slice(half * I, (half + 1) * I)
        maps.append({
            "xt": _pm(np.ascontiguousarray(hidden_states[b].T)),
            "wq": _pm(Wq[:, sl]),
            "wk": _pm(Wk[:, sl]),
            "wv": _pm(Wv[:, sl]),
            "wo": _pm(Wo[sl, :]),
        })
    return maps


@functools.lru_cache(maxsize=1)
def _runner():
    """Compile the SPMD program once and return a function
    maps -> list of per-core output dicts."""
    import jax
    from jax.sharding import Mesh, PartitionSpec, NamedSharding
    from jax.experimental.shard_map import shard_map

    import concourse.mybir as mybir
    from concourse.bass2jax import (
        _bass_exec_p, install_neuronx_cc_hook, partition_id_tensor)

    nc = _built()
    install_neuronx_cc_hook()
    partition_name = nc.partition_id_tensor.name if nc.partition_id_tensor else None

    in_names, out_names, out_avals, zero_outs = [], [], [], []
    for alloc in nc.m.functions[0].allocations:
        if not isinstance(alloc, mybir.MemoryLocationSet):
            continue
        name = alloc.memorylocations[0].name
        if alloc.kind == "ExternalInput":
            if name != partition_name:
                in_names.append(name)
        elif alloc.kind == "ExternalOutput":
            out_names.append(name)
            shape = tuple(alloc.tensor_shape)
            dtype = mybir.dt.np(alloc.dtype)
            out_avals.append(jax.core.ShapedArray(shape, dtype))
            zero_outs.append(np.zeros(shape, dtype))
    n_params = len(in_names)
    all_in_names = in_names + out_names
    if partition_name is not None:
        all_in_names = all_in_names + [partition_name]

    def _body(*args):
        operands = list(args)
        if partition_name is not None:
            operands.append(partition_id_tensor())
        return tuple(_bass_exec_p.bind(
            *operands,
            out_avals=tuple(out_avals),
            in_names=tuple(all_in_names),
            out_names=tuple(out_names),
            lowering_input_output_aliases=(),
            sim_require_finite=True,
            sim_require_nnan=True,
            nc=nc,
        ))

    devices = jax.devices()[:NCORES]
    mesh = Mesh(np.asarray(devices), ("core",))
    in_specs = (PartitionSpec("core"),) * (n_params + len(out_names))
    out_specs = (PartitionSpec("core"),) * len(out_names)
    sharded = jax.jit(
        shard_map(_body, mesh=mesh, in_specs=in_specs, out_specs=out_specs,
                  check_rep=False),
        keep_unused=True,
    )
    sharding = NamedSharding(mesh, PartitionSpec("core"))
    dev_zero = [jax.device_put(
        np.zeros((NCORES * z.shape[0], *z.shape[1:]), z.dtype), sharding)
        for z in zero_outs]

    def run(maps):
        concat_in = [np.concatenate([np.asarray(maps[c][n]) for c in range(NCORES)],
                                    axis=0) for n in in_names]
        dev_in = [jax.device_put(a, sharding) for a in concat_in]
        out_arrs = sharded(*dev_in, *dev_zero)
        return [
            {n: np.asarray(out_arrs[i]).reshape(NCORES, *out_avals[i].shape)[c]
             for i, n in enumerate(out_names)}
            for c in range(NCORES)
        ]

    return run


def kernel(hidden_states, Wq, Wk, Wv, Wo, bo):
    maps = _in_maps(np.asarray(hidden_states), np.asarray(Wq), np.asarray(Wk),
                    np.asarray(Wv), np.asarray(Wo))
    results = _runner()(maps)
    B = hidden_states.shape[0]
    out = np.empty((B, S, C), np.float32)
    for b in range(B):
        out[b] = (results[2 * b]["out"].astype(np.float32)
                  + results[2 * b + 1]["out"].astype(np.float32))
    out += np.asarray(bo, np.float32)
    return out


# revision 35
# speedup vs baseline: 1.1150x; 1.0388x over previous
"""Multi-head attention (B=4, S=2048, H=16, D=64, C=1024) on 8 NeuronCores.

Sharding: core c handles batch b=c//2 and head-half half=c%2 (8 heads = 512
inner dims).  Each core computes q/k/v projections for its half of the heads,
full softmax attention over S=2048, and a partial output projection through
its 512 rows of Wo.  Host sums the two partials per batch and adds the bias.

Per-core kernel layout (all matmul operands bf16, PSUM accumulation fp32):
  xt    [C=1024, S=2048]   hidden_states[b].T          (host pre-transposed)
  wq/wk/wv [C, I=512]      per-half weight columns
  wo    [I=512, C=1024]    per-half weight rows
  qT,kT [I, S] stored as 4 SBUF tiles [128, 2048]  (head pair per tile)
  v_pad [S, 8*65]          v with a ones column per head (row-sum via matmul)
  scores^T per (pair, qi-chunk, kj-tile): [kj=128, qi=512] via row-tiled
  (K=64) matmul pairs; exp on ScalarE; p@[v|1] accumulated in PSUM over kj.

v2 schedule (default flags "bcpsum,auxacc,pburst4,dmaq"):
  - constant memsets (v ones-columns) hoisted out of the repeat loop
    (a [128,8320] memset measures ~160us on HW);
  - one DMA per input tensor, all on the SP queue; output stores on the
    GPSIMD SWDGE queue so no dma_start wait can stall the ACT queue;
  - quarter-chunk p@v bursts: the scores->exp stream runs 4 kt ahead into
    an SBUF ring, then 8 pv matmuls burst, decoupling the in-order PE
    from the per-kt PE->ACT->PE round trip;
  - softmax normalization off the PSUM critical path: DVE evacuates the
    pv accumulators to SBUF immediately (banks recycle), then
    approx-reciprocal of the ones-column row-sum, GPSIMD
    partition_broadcast into SBUF (the idle engine; a PE K=1 broadcast
    matmul through PSUM measured 40us slower), and a DVE multiply;
  - scores PSUM ring is a pure 2-slot double buffer; proj/outproj
    accumulators get the third bank pair (sharing rings across
    consumer engines convoys the whole pipeline).
Measured ~445us per iteration vs 606us for the v1 baseline; relative
error 2.60e-3 on every compile draw.
"""

import functools

import numpy as np
import ml_dtypes

S = 2048          # sequence length
C = 1024          # query dim
I = 512           # inner dims per core (8 heads x 64)
HC = 8            # heads per core
D = 64            # head dim
NCORES = 8
SCALE = D ** -0.5
CT = C // 128     # 8 c-tiles
IT = I // 128     # 4 i-tiles (head pairs)
ST = S // 128     # 16 s-tiles
NQ = S // 512     # 4 qi chunks
VW = D + 1        # 65: v plus ones column


def _build(repeat=1, phases="dma,proj,attn,outproj,bcpsum,auxacc,pburst4,dmaq,gpbc", fused_exp=True):
    import contextlib

    import concourse.bacc as bacc
    import concourse.tile as tile
    from concourse import mybir

    f32 = mybir.dt.float32
    bf16 = mybir.dt.bfloat16
    fp8 = mybir.dt.float8e4
    DR = mybir.MatmulPerfMode.DoubleRow
    Exp = mybir.ActivationFunctionType.Exp

    nc = bacc.Bacc("TRN2", target_bir_lowering=False, debug=False,
                   num_devices=NCORES)

    # All inputs are host-pre-shuffled to partition-major [128, ...] layouts so
    # every DMA reads long contiguous runs per partition.
    xt_d = nc.dram_tensor("xt", [128, CT * S], bf16, kind="ExternalInput").ap()
    wq_d = nc.dram_tensor("wq", [128, CT * I], bf16, kind="ExternalInput").ap()
    wk_d = nc.dram_tensor("wk", [128, CT * I], bf16, kind="ExternalInput").ap()
    wv_d = nc.dram_tensor("wv", [128, CT * I], bf16, kind="ExternalInput").ap()
    wo_d = nc.dram_tensor("wo", [128, IT * C], bf16, kind="ExternalInput").ap()
    out_d = nc.dram_tensor("out", [S, C], bf16, kind="ExternalOutput").ap()

    with tile.TileContext(nc) as tc:
        with contextlib.ExitStack() as ctx:
            pflags = set(phases.split(","))
            ps3 = "ps3" in pflags and "gpbc" in pflags
            bc_own = "bcpsum" in pflags and not ps3
            aux_acc = "auxacc" in pflags
            const = ctx.enter_context(tc.tile_pool(name="const", bufs=1))
            work = ctx.enter_context(tc.tile_pool(name="work", bufs=4))
            outp = ctx.enter_context(tc.tile_pool(name="outp", bufs=4))
            ps_pool = ctx.enter_context(
                tc.tile_pool(name="ps", bufs=(2 if bc_own else 3), space="PSUM"))
            # with gpbc+ps3 the broadcast lives in SBUF: scores get 3 slots
            # and the proj/outproj accumulators ride the ring's spare slot.
            pv_pool = ctx.enter_context(tc.tile_pool(name="pv", bufs=2, space="PSUM"))
            bc_pool = ctx.enter_context(
                tc.tile_pool(name="bcp", bufs=1, space="PSUM")) if bc_own else None
            # proj/outproj accumulators share the bc banks (used only at chunk
            # ends) so the scores ring stays a pure double-buffer.
            acc_pool = bc_pool if (bc_own and aux_acc) else ps_pool
            acc_tag = "bc" if (bc_own and aux_acc) else "ps"

            # ---- persistent tiles -------------------------------------------
            xt_sb = const.tile([128, CT, S], bf16)
            wq_sb = const.tile([128, CT, I], bf16)
            wk_sb = const.tile([128, CT, I], bf16)
            wv_sb = const.tile([128, CT, I], bf16)
            wo_sb = const.tile([128, IT, C], bf16)
            qT_sb = const.tile([128, IT, S], bf16)
            kT_sb = const.tile([128, IT, S], bf16)
            pv8 = "pv8" in pflags
            if not pv8:
                v_sb = const.tile([128, ST, HC * VW], bf16)
            VW8 = 80
            if pv8:
                v8_sb = const.tile([128, ST // 2, 2, HC, VW8], fp8)
            oT_sb = const.tile([128, IT, S], bf16)
            ones64 = const.tile([1, D], f32)
            ones64h = const.tile([1, D], bf16)

            # ---- one-time setup (outside the repeat loop) -------------------
            # ones columns per head (softmax denominator): contiguous memset of
            # the whole tile — the v copies then overwrite the 64 data columns.
            # This memset measures ~160us on HW, so it must stay out of the
            # steady-state loop.
            if pv8:
                nc.vector.memset(v8_sb, 1.0)
            else:
                nc.vector.memset(v_sb, 1.0)
            nc.vector.memset(ones64, 1.0)
            nc.vector.memset(ones64h, 1.0)

            preproj = "preproj" in pflags
            loop_entered = []

            def enter_loop():
                if repeat > 1:
                    ctx.enter_context(tc.For_i(0, repeat, 1))
                loop_entered.append(True)

            if not preproj:
                enter_loop()

            # ---- input DMAs (one per tensor, contiguous per partition) ------
            # All on the sync (SP) queue: a dma_start's wait blocks its whole
            # issuing queue, and the ACT queue must never stall (exp stream).
            dmaq = "dmaq" in pflags
            wq_eng = nc.sync if dmaq else nc.scalar

            def input_dmas(with_wo=True):
                nc.sync.dma_start(out=xt_sb,
                                  in_=xt_d.rearrange("p (t s) -> p t s", s=S))
                wq_eng.dma_start(out=wq_sb,
                                 in_=wq_d.rearrange("p (t i) -> p t i", i=I))
                wq_eng.dma_start(out=wk_sb,
                                 in_=wk_d.rearrange("p (t i) -> p t i", i=I))
                wq_eng.dma_start(out=wv_sb,
                                 in_=wv_d.rearrange("p (t i) -> p t i", i=I))
                if with_wo:
                    wq_eng.dma_start(out=wo_sb,
                                     in_=wo_d.rearrange("p (t c) -> p t c", c=C))

            input_dmas()

            phs = set(phases.split(","))

            # ---- projections -------------------------------------------------
            if "proj" in phs:
                def proj_qk_acc(it, which, nq):
                    w_sb, o_sb = ((wq_sb, qT_sb), (wk_sb, kT_sb))[which]
                    acc = acc_pool.tile([128, 1024], f32, tag=acc_tag,
                                        name="proj_ps")
                    for ct in range(CT):
                        for h2 in range(2):
                            nc.tensor.matmul(
                                acc[:, h2 * 512:(h2 + 1) * 512],
                                lhsT=w_sb[:, ct, it * 128:(it + 1) * 128],
                                rhs=xt_sb[:, ct, nq * 1024 + h2 * 512:
                                          nq * 1024 + (h2 + 1) * 512],
                                start=(ct == 0), stop=(ct == CT - 1))
                    nc.vector.tensor_copy(
                        out=o_sb[:, it, nq * 1024:(nq + 1) * 1024], in_=acc)

                def proj_qk(it):
                    for which in range(2):
                        for nq in range(S // 1024):
                            proj_qk_acc(it, which, nq)

                v_main = (None if pv8 else v_sb.rearrange(
                    "p t (h e) -> p t h e", e=VW)[:, :, :, 0:D])

                def proj_v(st):
                    acc = acc_pool.tile([128, 1024], f32, tag=acc_tag,
                                        name="v_ps")[:, 0:512]
                    for ct in range(CT):
                        nc.tensor.matmul(
                            acc,
                            lhsT=xt_sb[:, ct, st * 128:(st + 1) * 128],
                            rhs=wv_sb[:, ct, :],
                            start=(ct == 0), stop=(ct == CT - 1))
                    if pv8:
                        nc.vector.tensor_copy(
                            out=v8_sb[:, st // 2, st % 2, :, 0:D],
                            in_=acc.rearrange("p (h d) -> p h d", d=D))
                    else:
                        nc.vector.tensor_copy(
                            out=v_main[:, st],
                            in_=acc.rearrange("p (h d) -> p h d", d=D))

            # ---- attention + interleaved output projection ------------------
            def attn_chunk(hp, nq, extra=None):
                hA, hB = 2 * hp, 2 * hp + 1
                qs = slice(nq * 512, (nq + 1) * 512)
                oA = pv_pool.tile([VW, 512], f32, tag="pv", name="oA")
                oB = pv_pool.tile([VW, 512], f32, tag="pv", name="oB")
                if "pburst4" in phs:
                    # bf16 quarter-chunk bursts: scores+exp stream ahead, pv
                    # runs as 8-matmul bursts every 4 kt — decouples the
                    # in-order PE from the per-kt ACT round trip without fp8.
                    for quarter in range(4):
                        ph = work.tile([128, 4, 1024], bf16, tag="p",
                                       bufs=(3 if "ph3" in phs else 2),
                                       name="ph")
                        for k4 in range(4):
                            kt = quarter * 4 + k4
                            if extra is not None:
                                extra(kt)
                            ks = slice(kt * 128, (kt + 1) * 128)
                            sAB = ps_pool.tile([128, 1024], f32, tag="ps",
                                               name="sAB")
                            nc.tensor.matmul(
                                sAB[:, 0:512], lhsT=kT_sb[0:64, hp, ks],
                                rhs=qT_sb[0:64, hp, qs],
                                start=True, stop=True, tile_position=(0, 0))
                            nc.tensor.matmul(
                                sAB[:, 512:1024], lhsT=kT_sb[64:128, hp, ks],
                                rhs=qT_sb[64:128, hp, qs],
                                start=True, stop=True, tile_position=(64, 0))
                            nc.scalar.activation(out=ph[:, k4, :], in_=sAB,
                                                 func=Exp, scale=SCALE)
                        for k4 in range(4):
                            kt = quarter * 4 + k4
                            nc.tensor.matmul(
                                oA, lhsT=v_sb[:, kt, hA * VW:(hA + 1) * VW],
                                rhs=ph[:, k4, 0:512],
                                start=(kt == 0), stop=(kt == ST - 1))
                            nc.tensor.matmul(
                                oB, lhsT=v_sb[:, kt, hB * VW:(hB + 1) * VW],
                                rhs=ph[:, k4, 512:1024],
                                start=(kt == 0), stop=(kt == ST - 1))
                elif pv8 and "pburst" in phs:
                    # whole-chunk p buffer: scores+exp stream fully decoupled
                    # from the pv burst, which pipelines against the NEXT
                    # chunk's exp stream at chunk granularity.
                    pc = work.tile([128, ST // 2, 2, 1024], fp8, tag="pc",
                                   bufs=2, name="pc")
                    for kt in range(ST):
                        if extra is not None:
                            extra(kt)
                        ks = slice(kt * 128, (kt + 1) * 128)
                        sAB = ps_pool.tile([128, 1024], f32, tag="ps",
                                           name="sAB")
                        nc.tensor.matmul(
                            sAB[:, 0:512], lhsT=kT_sb[0:64, hp, ks],
                            rhs=qT_sb[0:64, hp, qs],
                            start=True, stop=True, tile_position=(0, 0))
                        nc.tensor.matmul(
                            sAB[:, 512:1024], lhsT=kT_sb[64:128, hp, ks],
                            rhs=qT_sb[64:128, hp, qs],
                            start=True, stop=True, tile_position=(64, 0))
                        nc.scalar.activation(out=pc[:, kt // 2, kt % 2, :],
                                             in_=sAB, func=Exp, scale=SCALE)
                    for ktp in range(ST // 2):
                        nc.tensor.matmul(
                            oA, lhsT=v8_sb[:, ktp, :, hA, 0:VW],
                            rhs=pc[:, ktp, :, 0:512],
                            start=(ktp == 0), stop=(ktp == ST // 2 - 1),
                            perf_mode=DR)
                        nc.tensor.matmul(
                            oB, lhsT=v8_sb[:, ktp, :, hB, 0:VW],
                            rhs=pc[:, ktp, :, 512:1024],
                            start=(ktp == 0), stop=(ktp == ST // 2 - 1),
                            perf_mode=DR)
                elif pv8:
                    for ktp in range(ST // 2):
                        p2 = work.tile([128, 2, 1024], fp8, tag="p", bufs=4,
                                       name="p2")
                        for j in range(2):
                            kt = 2 * ktp + j
                            if extra is not None:
                                extra(kt)
                            ks = slice(kt * 128, (kt + 1) * 128)
                            sAB = ps_pool.tile([128, 1024], f32, tag="ps",
                                               name="sAB")
                            nc.tensor.matmul(
                                sAB[:, 0:512], lhsT=kT_sb[0:64, hp, ks],
                                rhs=qT_sb[0:64, hp, qs],
                                start=True, stop=True, tile_position=(0, 0))
                            nc.tensor.matmul(
                                sAB[:, 512:1024], lhsT=kT_sb[64:128, hp, ks],
                                rhs=qT_sb[64:128, hp, qs],
                                start=True, stop=True, tile_position=(64, 0))
                            nc.scalar.activation(out=p2[:, j, :], in_=sAB,
                                                 func=Exp, scale=SCALE)
                        nc.tensor.matmul(
                            oA, lhsT=v8_sb[:, ktp, :, hA, 0:VW],
                            rhs=p2[:, :, 0:512],
                            start=(ktp == 0), stop=(ktp == ST // 2 - 1),
                            perf_mode=DR)
                        nc.tensor.matmul(
                            oB, lhsT=v8_sb[:, ktp, :, hB, 0:VW],
                            rhs=p2[:, :, 512:1024],
                            start=(ktp == 0), stop=(ktp == ST // 2 - 1),
                            perf_mode=DR)
                else:
                  for kt in range(ST):
                    if extra is not None:
                        extra(kt)
                    ks = slice(kt * 128, (kt + 1) * 128)
                    sAB = ps_pool.tile([128, 1024], f32, tag="ps", name="sAB")
                    sA, sB = sAB[:, 0:512], sAB[:, 512:1024]
                    nc.tensor.matmul(
                        sA, lhsT=kT_sb[0:64, hp, ks], rhs=qT_sb[0:64, hp, qs],
                        start=True, stop=True, tile_position=(0, 0))
                    nc.tensor.matmul(
                        sB, lhsT=kT_sb[64:128, hp, ks], rhs=qT_sb[64:128, hp, qs],
                        start=True, stop=True, tile_position=(64, 0))
                    pAB = work.tile([128, 1024], bf16, tag="p", bufs=8, name="pAB")
                    if "act512" in phs:
                        nc.scalar.activation(out=pAB[:, 0:512], in_=sA,
                                             func=Exp, scale=SCALE)
                        nc.scalar.activation(out=pAB[:, 512:1024], in_=sB,
                                             func=Exp, scale=SCALE)
                    else:
                        nc.scalar.activation(out=pAB, in_=sAB, func=Exp, scale=SCALE)
                    pvn = 256 if "pvhalf" in phs else 512
                    nc.tensor.matmul(
                        oA[:, 0:pvn], lhsT=v_sb[:, kt, hA * VW:(hA + 1) * VW],
                        rhs=pAB[:, 0:pvn],
                        start=(kt == 0), stop=(kt == ST - 1))
                    nc.tensor.matmul(
                        oB[:, 0:pvn], lhsT=v_sb[:, kt, hB * VW:(hB + 1) * VW],
                        rhs=pAB[:, 512:512 + pvn],
                        start=(kt == 0), stop=(kt == ST - 1))
                if "nonorm" in phs:
                    return
                # Normalization, off the PSUM critical path: evacuate oA/oB to
                # SBUF immediately (frees the pv banks for the next chunk),
                # then 1/denominator (approx, 18-bit), broadcast across the 64
                # head dims via a K=1 matmul, and multiply on DVE.
                oab = work.tile([VW, 1024], bf16, tag="oab", bufs=3, name="oab")
                nc.vector.tensor_copy(out=oab[:, 0:512], in_=oA)
                nc.vector.tensor_copy(out=oab[:, 512:1024], in_=oB)
                if "oldnorm" in phs:
                    rA = work.tile([1, 512], f32, tag="recip", bufs=4, name="rA")
                    rB = work.tile([1, 512], f32, tag="recip", bufs=4, name="rB")
                    nc.vector.reciprocal(out=rA, in_=oab[D:VW, 0:512])
                    nc.vector.reciprocal(out=rB, in_=oab[D:VW, 512:1024])
                    bcA = work.tile([64, 512], f32, tag="bcs", bufs=2, name="bcA")
                    bcB = work.tile([64, 512], f32, tag="bcs", bufs=2, name="bcB")
                    nc.gpsimd.partition_broadcast(bcA, rA)
                    nc.gpsimd.partition_broadcast(bcB, rB)
                    nc.vector.tensor_mul(
                        out=oT_sb[0:64, hp, qs], in0=oab[0:D, 0:512], in1=bcA)
                    nc.vector.tensor_mul(
                        out=oT_sb[64:128, hp, qs], in0=oab[0:D, 512:1024], in1=bcB)
                    return
                # the custom recip op only works at base partition 0, so plain-
                # copy the denominator row down to partition 0 first.
                oden = work.tile([1, 1024], f32, tag="oden", bufs=2, name="oden")
                nc.vector.tensor_copy(out=oden, in_=oab[D:VW, :])
                rAB = work.tile([1, 1024], f32, tag="recip", bufs=2, name="rAB")
                nc.vector.reciprocal_approx_fast(out=rAB, in_=oden)
                if "gpbc" in phs:
                    bcg = work.tile([D, 1024], f32, tag="bcg", bufs=2,
                                    name="bcg")
                    nc.gpsimd.partition_broadcast(bcg, rAB)
                    nc.vector.tensor_mul(
                        out=oT_sb[0:64, hp, qs], in0=oab[0:D, 0:512],
                        in1=bcg[:, 0:512])
                    nc.vector.tensor_mul(
                        out=oT_sb[64:128, hp, qs], in0=oab[0:D, 512:1024],
                        in1=bcg[:, 512:1024])
                    return
                rABh = work.tile([1, 1024], bf16, tag="reciph", bufs=2,
                                 name="rABh")
                nc.vector.tensor_copy(out=rABh, in_=rAB)
                if bc_own:
                    bc = bc_pool.tile([128, 1024], f32, tag="bc", name="bc")
                else:
                    bc = ps_pool.tile([128, 1024], f32, tag="ps", name="bc")
                nc.tensor.matmul(bc[0:D, 0:512], lhsT=ones64h,
                                 rhs=rABh[:, 0:512], start=True, stop=True)
                nc.tensor.matmul(bc[0:D, 512:1024], lhsT=ones64h,
                                 rhs=rABh[:, 512:1024], start=True, stop=True)
                nc.vector.tensor_mul(
                    out=oT_sb[0:64, hp, qs], in0=oab[0:D, 0:512], in1=bc[0:D, 0:512])
                nc.vector.tensor_mul(
                    out=oT_sb[64:128, hp, qs], in0=oab[0:D, 512:1024],
                    in1=bc[0:D, 512:1024])

            def outproj_tile(st):
                acc = acc_pool.tile([128, 1024], f32, tag=acc_tag,
                                    name="out_ps")
                for it in range(IT):
                    for h2 in range(2):
                        nc.tensor.matmul(
                            acc[:, h2 * 512:(h2 + 1) * 512],
                            lhsT=oT_sb[:, it, st * 128:(st + 1) * 128],
                            rhs=wo_sb[:, it, h2 * 512:(h2 + 1) * 512],
                            start=(it == 0), stop=(it == IT - 1))
                ob = outp.tile([128, 1024], bf16, tag="ob", bufs=3, name="ob")
                nc.vector.tensor_copy(out=ob, in_=acc)
                (nc.gpsimd if dmaq else nc.sync).dma_start(
                    out=out_d[st * 128:(st + 1) * 128, :], in_=ob)

            if "attn" in phs:
                # pair-major chunk order; qk projections for the next pair and
                # v tiles are woven into chunk kt-steps so PE slack under the
                # ACT-bound exp stream absorbs them.
                if "proj" in phs:
                    if "noweave" in phs:
                        if preproj:
                            enter_loop()
                        for it in range(IT):
                            proj_qk(it)
                        for st in range(ST):
                            proj_v(st)
                    elif preproj:
                        # first-iteration qk(0) runs once, outside the loop;
                        # steady-state iterations get it from the pair-3 weave.
                        proj_qk(0)
                        enter_loop()
                        input_dmas()
                        for st in range(4):
                            proj_v(st)
                    else:
                        proj_qk(0)
                        for st in range(4):
                            proj_v(st)

                def make_extra(hp, nq):
                    if "proj" not in phs or "noweave" in phs:
                        return None
                    def extra(kt):
                        if hp == 0 and nq == 0 and 4 + kt < ST:
                            proj_v(4 + kt)
                        wrap = preproj and hp == IT - 1
                        if (hp < IT - 1 or wrap) and nq in (1, 2) and kt in (3, 11):
                            acc_idx = (nq - 1) * 2 + (0 if kt == 3 else 1)
                            proj_qk_acc((hp + 1) % IT, acc_idx // 2, acc_idx % 2)
                    return extra

                for hp in range(IT):
                    for nq in range(NQ):
                        attn_chunk(hp, nq, make_extra(hp, nq))
                        if (hp == IT - 1 and "outproj" in phs
                                and "outlate" not in phs):
                            for st in range(4 * nq, 4 * (nq + 1)):
                                outproj_tile(st)
                if "outproj" in phs and "outlate" in phs:
                    for st in range(ST):
                        outproj_tile(st)
            else:
                if preproj:
                    enter_loop()
                if "proj" in phs:
                    for it in range(IT):
                        proj_qk(it)
                    for st in range(ST):
                        proj_v(st)
                if "outproj" in phs:
                    for st in range(ST):
                        outproj_tile(st)

    nc.compile()
    return nc


@functools.lru_cache(maxsize=8)
def _built(repeat=1, phases="dma,proj,attn,outproj,bcpsum,auxacc,pburst4,dmaq,gpbc", fused_exp=True):
    return _build(repeat, phases, fused_exp)


def _pm(a):
    """[T*128, F] -> partition-major [128, T*F] (bf16)."""
    T = a.shape[0] // 128
    return np.ascontiguousarray(
        a.reshape(T, 128, a.shape[1]).swapaxes(0, 1).reshape(128, -1)
    ).astype(ml_dtypes.bfloat16)


def _in_maps(hidden_states, Wq, Wk, Wv, Wo):
    maps = []
    for c in range(NCORES):
        b, half = divmod(c, 2)
        sl = slice(half * I, (half + 1) * I)
        maps.append({
            "xt": _pm(np.ascontiguousarray(hidden_states[b].T)),
            "wq": _pm(Wq[:, sl]),
            "wk": _pm(Wk[:, sl]),
            "wv": _pm(Wv[:, sl]),
            "wo": _pm(Wo[sl, :]),
        })
    return maps


@functools.lru_cache(maxsize=1)
def _runner():
    """Compile the SPMD program once and return a function
    maps -> list of per-core output dicts."""
    import jax
    from jax.sharding import Mesh, PartitionSpec, NamedSharding
    from jax.experimental.shard_map import shard_map

    import concourse.mybir as mybir
    from concourse.bass2jax import (
        _bass_exec_p, install_neuronx_cc_hook, partition_id_tensor)

    nc = _built()
    install_neuronx_cc_hook()
    partition_name = nc.partition_id_tensor.name if nc.partition_id_tensor else None

    in_names, out_names, out_avals, zero_outs = [], [], [], []
    for alloc in nc.m.functions[0].allocations:
        if not isinstance(alloc, mybir.MemoryLocationSet):
            continue
        name = alloc.memorylocations[0].name
        if alloc.kind == "ExternalInput":
            if name != partition_name:
                in_names.append(name)
        elif alloc.kind == "ExternalOutput":
            out_names.append(name)
            shape = tuple(alloc.tensor_shape)
            dtype = mybir.dt.np(alloc.dtype)
            out_avals.append(jax.core.ShapedArray(shape, dtype))
            zero_outs.append(np.zeros(shape, dtype))
    n_params = len(in_names)
    all_in_names = in_names + out_names
    if partition_name is not None:
        all_in_names = all_in_names + [partition_name]

    def _body(*args):
        operands = list(args)
        if partition_name is not None:
            operands.append(partition_id_tensor())
        return tuple(_bass_exec_p.bind(
            *operands,
            out_avals=tuple(out_avals),
            in_names=tuple(all_in_names),
            out_names=tuple(out_names),
            lowering_input_output_aliases=(),
            sim_require_finite=True,
            sim_require_nnan=True,
            nc=nc,
        ))

    devices = jax.devices()[:NCORES]
    mesh = Mesh(np.asarray(devices), ("core",))
    in_specs = (PartitionSpec("core"),) * (n_params + len(out_names))
    out_specs = (PartitionSpec("core"),) * len(out_names)
    sharded = jax.jit(
        shard_map(_body, mesh=mesh, in_specs=in_specs, out_specs=out_specs,
                  check_rep=False),
        keep_unused=True,
    )
    sharding = NamedSharding(mesh, PartitionSpec("core"))
    dev_zero = [jax.device_put(
        np.zeros((NCORES * z.shape[0], *z.shape[1:]), z.dtype), sharding)
        for z in zero_outs]

    def run(maps):
        concat_in = [np.concatenate([np.asarray(maps[c][n]) for c in range(NCORES)],
                                    axis=0) for n in in_names]
        dev_in = [jax.device_put(a, sharding) for a in concat_in]
        out_arrs = sharded(*dev_in, *dev_zero)
        return [
            {n: np.asarray(out_arrs[i]).reshape(NCORES, *out_avals[i].shape)[c]
             for i, n in enumerate(out_names)}
            for c in range(NCORES)
        ]

    return run


def kernel(hidden_states, Wq, Wk, Wv, Wo, bo):
    maps = _in_maps(np.asarray(hidden_states), np.asarray(Wq), np.asarray(Wk),
                    np.asarray(Wv), np.asarray(Wo))
    results = _runner()(maps)
    B = hidden_states.shape[0]
    out = np.empty((B, S, C), np.float32)
    for b in range(B):
        out[b] = (results[2 * b]["out"].astype(np.float32)
                  + results[2 * b + 1]["out"].astype(np.float32))
    out += np.asarray(bo, np.float32)
    return out


# revision 38
# speedup vs baseline: 1.1225x; 1.0068x over previous
"""Multi-head attention (B=4, S=2048, H=16, D=64, C=1024) on 8 NeuronCores.

Sharding: core c handles batch b=c//2 and head-half half=c%2 (8 heads = 512
inner dims).  Each core computes q/k/v projections for its half of the heads,
full softmax attention over S=2048, and a partial output projection through
its 512 rows of Wo.  Host sums the two partials per batch and adds the bias.

Per-core kernel layout (all matmul operands bf16, PSUM accumulation fp32):
  xt    [C=1024, S=2048]   hidden_states[b].T          (host pre-transposed)
  wq/wk/wv [C, I=512]      per-half weight columns
  wo    [I=512, C=1024]    per-half weight rows
  qT,kT [I, S] stored as 4 SBUF tiles [128, 2048]  (head pair per tile)
  v_pad [S, 8*65]          v with a ones column per head (row-sum via matmul)
  scores^T per (pair, qi-chunk, kj-tile): [kj=128, qi=512] via row-tiled
  (K=64) matmul pairs; exp on ScalarE; p@[v|1] accumulated in PSUM over kj.

v2 schedule (default flags "bcpsum,auxacc,pburst4,dmaq"):
  - constant memsets (v ones-columns) hoisted out of the repeat loop
    (a [128,8320] memset measures ~160us on HW);
  - one DMA per input tensor, all on the SP queue; output stores on the
    GPSIMD SWDGE queue so no dma_start wait can stall the ACT queue;
  - quarter-chunk p@v bursts: the scores->exp stream runs 4 kt ahead into
    an SBUF ring, then 8 pv matmuls burst, decoupling the in-order PE
    from the per-kt PE->ACT->PE round trip;
  - softmax normalization off the PSUM critical path: DVE evacuates the
    pv accumulators to SBUF immediately (banks recycle), then
    approx-reciprocal of the ones-column row-sum, GPSIMD
    partition_broadcast into SBUF (the idle engine; a PE K=1 broadcast
    matmul through PSUM measured 40us slower), and a DVE multiply;
  - scores PSUM ring is a pure 2-slot double buffer; proj/outproj
    accumulators get the third bank pair (sharing rings across
    consumer engines convoys the whole pipeline).
Measured ~445us per iteration vs 606us for the v1 baseline; relative
error 2.60e-3 on every compile draw.
"""

import functools

import numpy as np
import ml_dtypes

S = 2048          # sequence length
C = 1024          # query dim
I = 512           # inner dims per core (8 heads x 64)
HC = 8            # heads per core
D = 64            # head dim
NCORES = 8
SCALE = D ** -0.5
CT = C // 128     # 8 c-tiles
IT = I // 128     # 4 i-tiles (head pairs)
ST = S // 128     # 16 s-tiles
NQ = S // 512     # 4 qi chunks
VW = D + 1        # 65: v plus ones column


def _build(repeat=1, phases="dma,proj,attn,outproj,bcpsum,auxacc,pburst4,dmaq,gpbc", fused_exp=True):
    import contextlib

    import concourse.bacc as bacc
    import concourse.tile as tile
    from concourse import mybir

    f32 = mybir.dt.float32
    bf16 = mybir.dt.bfloat16
    fp8 = mybir.dt.float8e4
    DR = mybir.MatmulPerfMode.DoubleRow
    Exp = mybir.ActivationFunctionType.Exp

    nc = bacc.Bacc("TRN2", target_bir_lowering=False, debug=False,
                   num_devices=NCORES)

    # All inputs are host-pre-shuffled to partition-major [128, ...] layouts so
    # every DMA reads long contiguous runs per partition.
    xt_d = nc.dram_tensor("xt", [128, CT * S], bf16, kind="ExternalInput").ap()
    wq_d = nc.dram_tensor("wq", [128, CT * I], bf16, kind="ExternalInput").ap()
    wk_d = nc.dram_tensor("wk", [128, CT * I], bf16, kind="ExternalInput").ap()
    wv_d = nc.dram_tensor("wv", [128, CT * I], bf16, kind="ExternalInput").ap()
    wo_d = nc.dram_tensor("wo", [128, IT * C], bf16, kind="ExternalInput").ap()
    out_d = nc.dram_tensor("out", [S, C], bf16, kind="ExternalOutput").ap()

    with tile.TileContext(nc) as tc:
        with contextlib.ExitStack() as ctx:
            pflags = set(phases.split(","))
            ps3 = "ps3" in pflags and "gpbc" in pflags
            bc_own = "bcpsum" in pflags and not ps3
            aux_acc = "auxacc" in pflags
            const = ctx.enter_context(tc.tile_pool(name="const", bufs=1))
            work = ctx.enter_context(tc.tile_pool(name="work", bufs=4))
            outp = ctx.enter_context(tc.tile_pool(name="outp", bufs=4))
            ps_pool = ctx.enter_context(
                tc.tile_pool(name="ps", bufs=(2 if bc_own else 3), space="PSUM"))
            # with gpbc+ps3 the broadcast lives in SBUF: scores get 3 slots
            # and the proj/outproj accumulators ride the ring's spare slot.
            pv_pool = ctx.enter_context(tc.tile_pool(name="pv", bufs=2, space="PSUM"))
            bc_pool = ctx.enter_context(
                tc.tile_pool(name="bcp", bufs=1, space="PSUM")) if bc_own else None
            # proj/outproj accumulators share the bc banks (used only at chunk
            # ends) so the scores ring stays a pure double-buffer.
            acc_pool = bc_pool if (bc_own and aux_acc) else ps_pool
            acc_tag = "bc" if (bc_own and aux_acc) else "ps"

            # ---- persistent tiles -------------------------------------------
            xt_sb = const.tile([128, CT, S], bf16)
            wq_sb = const.tile([128, CT, I], bf16)
            wk_sb = const.tile([128, CT, I], bf16)
            wv_sb = const.tile([128, CT, I], bf16)
            wo_sb = const.tile([128, IT, C], bf16)
            qT_sb = const.tile([128, IT, S], bf16)
            kT_sb = const.tile([128, IT, S], bf16)
            pv8 = "pv8" in pflags
            if not pv8:
                v_sb = const.tile([128, ST, HC * VW], bf16)
            VW8 = 80
            if pv8:
                v8_sb = const.tile([128, ST // 2, 2, HC, VW8], fp8)
            oT_sb = const.tile([128, IT, S], bf16)
            ones64 = const.tile([1, D], f32)
            ones64h = const.tile([1, D], bf16)

            # ---- one-time setup (outside the repeat loop) -------------------
            # ones columns per head (softmax denominator): contiguous memset of
            # the whole tile — the v copies then overwrite the 64 data columns.
            # This memset measures ~160us on HW, so it must stay out of the
            # steady-state loop.
            if pv8:
                nc.vector.memset(v8_sb, 1.0)
            else:
                nc.vector.memset(v_sb, 1.0)
            nc.vector.memset(ones64, 1.0)
            nc.vector.memset(ones64h, 1.0)

            preproj = "preproj" in pflags
            loop_entered = []

            def enter_loop():
                if repeat > 1:
                    ctx.enter_context(tc.For_i(0, repeat, 1))
                loop_entered.append(True)

            if not preproj:
                enter_loop()

            # ---- input DMAs (one per tensor, contiguous per partition) ------
            # All on the sync (SP) queue: a dma_start's wait blocks its whole
            # issuing queue, and the ACT queue must never stall (exp stream).
            dmaq = "dmaq" in pflags
            wq_eng = nc.sync if dmaq else nc.scalar

            def input_dmas(with_wo=True):
                nc.sync.dma_start(out=xt_sb,
                                  in_=xt_d.rearrange("p (t s) -> p t s", s=S))
                wq_eng.dma_start(out=wq_sb,
                                 in_=wq_d.rearrange("p (t i) -> p t i", i=I))
                wq_eng.dma_start(out=wk_sb,
                                 in_=wk_d.rearrange("p (t i) -> p t i", i=I))
                wq_eng.dma_start(out=wv_sb,
                                 in_=wv_d.rearrange("p (t i) -> p t i", i=I))
                if with_wo:
                    wq_eng.dma_start(out=wo_sb,
                                     in_=wo_d.rearrange("p (t c) -> p t c", c=C))

            input_dmas()

            phs = set(phases.split(","))

            # ---- projections -------------------------------------------------
            if "proj" in phs:
                def proj_qk_acc(it, which, nq):
                    w_sb, o_sb = ((wq_sb, qT_sb), (wk_sb, kT_sb))[which]
                    acc = acc_pool.tile([128, 1024], f32, tag=acc_tag,
                                        name="proj_ps")
                    for ct in range(CT):
                        for h2 in range(2):
                            nc.tensor.matmul(
                                acc[:, h2 * 512:(h2 + 1) * 512],
                                lhsT=w_sb[:, ct, it * 128:(it + 1) * 128],
                                rhs=xt_sb[:, ct, nq * 1024 + h2 * 512:
                                          nq * 1024 + (h2 + 1) * 512],
                                start=(ct == 0), stop=(ct == CT - 1))
                    nc.vector.tensor_copy(
                        out=o_sb[:, it, nq * 1024:(nq + 1) * 1024], in_=acc)

                def proj_qk(it):
                    for which in range(2):
                        for nq in range(S // 1024):
                            proj_qk_acc(it, which, nq)

                v_main = (None if pv8 else v_sb.rearrange(
                    "p t (h e) -> p t h e", e=VW)[:, :, :, 0:D])

                def proj_v(st):
                    acc = acc_pool.tile([128, 1024], f32, tag=acc_tag,
                                        name="v_ps")[:, 0:512]
                    for ct in range(CT):
                        nc.tensor.matmul(
                            acc,
                            lhsT=xt_sb[:, ct, st * 128:(st + 1) * 128],
                            rhs=wv_sb[:, ct, :],
                            start=(ct == 0), stop=(ct == CT - 1))
                    if pv8:
                        nc.vector.tensor_copy(
                            out=v8_sb[:, st // 2, st % 2, :, 0:D],
                            in_=acc.rearrange("p (h d) -> p h d", d=D))
                    else:
                        nc.vector.tensor_copy(
                            out=v_main[:, st],
                            in_=acc.rearrange("p (h d) -> p h d", d=D))

            # ---- attention + interleaved output projection ------------------
            def attn_chunk(hp, nq, extra=None):
                hA, hB = 2 * hp, 2 * hp + 1
                qs = slice(nq * 512, (nq + 1) * 512)
                oA = pv_pool.tile([VW, 512], f32, tag="pv", name="oA")
                oB = pv_pool.tile([VW, 512], f32, tag="pv", name="oB")
                if "pburst4" in phs:
                    # bf16 quarter-chunk bursts: scores+exp stream ahead, pv
                    # runs as 8-matmul bursts every 4 kt — decouples the
                    # in-order PE from the per-kt ACT round trip without fp8.
                    # With "skew", quarter q's burst is traced AFTER quarter
                    # q+1's scores so the in-order PE never sits waiting for
                    # the exp of the quarter it just scored.
                    skew = "skew" in phs
                    phq = [None] * 4

                    def scores_quarter(quarter):
                        ph = work.tile([128, 4, 1024], bf16, tag="p",
                                       bufs=(3 if "ph3" in phs else 2),
                                       name="ph")
                        phq[quarter] = ph
                        for k4 in range(4):
                            kt = quarter * 4 + k4
                            if extra is not None:
                                extra(kt)
                            ks = slice(kt * 128, (kt + 1) * 128)
                            sAB = ps_pool.tile([128, 1024], f32, tag="ps",
                                               name="sAB")
                            nc.tensor.matmul(
                                sAB[:, 0:512], lhsT=kT_sb[0:64, hp, ks],
                                rhs=qT_sb[0:64, hp, qs],
                                start=True, stop=True, tile_position=(0, 0))
                            nc.tensor.matmul(
                                sAB[:, 512:1024], lhsT=kT_sb[64:128, hp, ks],
                                rhs=qT_sb[64:128, hp, qs],
                                start=True, stop=True, tile_position=(64, 0))
                            nc.scalar.activation(out=ph[:, k4, :], in_=sAB,
                                                 func=Exp, scale=SCALE)

                    def burst_quarter(quarter):
                        ph = phq[quarter]
                        for k4 in range(4):
                            kt = quarter * 4 + k4
                            nc.tensor.matmul(
                                oA, lhsT=v_sb[:, kt, hA * VW:(hA + 1) * VW],
                                rhs=ph[:, k4, 0:512],
                                start=(kt == 0), stop=(kt == ST - 1))
                            nc.tensor.matmul(
                                oB, lhsT=v_sb[:, kt, hB * VW:(hB + 1) * VW],
                                rhs=ph[:, k4, 512:1024],
                                start=(kt == 0), stop=(kt == ST - 1))

                    if skew:
                        for quarter in range(5):
                            if quarter < 4:
                                scores_quarter(quarter)
                            if quarter >= 1:
                                burst_quarter(quarter - 1)
                    else:
                        for quarter in range(4):
                            scores_quarter(quarter)
                            burst_quarter(quarter)
                elif pv8 and "pburst" in phs:
                    # whole-chunk p buffer: scores+exp stream fully decoupled
                    # from the pv burst, which pipelines against the NEXT
                    # chunk's exp stream at chunk granularity.
                    pc = work.tile([128, ST // 2, 2, 1024], fp8, tag="pc",
                                   bufs=2, name="pc")
                    for kt in range(ST):
                        if extra is not None:
                            extra(kt)
                        ks = slice(kt * 128, (kt + 1) * 128)
                        sAB = ps_pool.tile([128, 1024], f32, tag="ps",
                                           name="sAB")
                        nc.tensor.matmul(
                            sAB[:, 0:512], lhsT=kT_sb[0:64, hp, ks],
                            rhs=qT_sb[0:64, hp, qs],
                            start=True, stop=True, tile_position=(0, 0))
                        nc.tensor.matmul(
                            sAB[:, 512:1024], lhsT=kT_sb[64:128, hp, ks],
                            rhs=qT_sb[64:128, hp, qs],
                            start=True, stop=True, tile_position=(64, 0))
                        nc.scalar.activation(out=pc[:, kt // 2, kt % 2, :],
                                             in_=sAB, func=Exp, scale=SCALE)
                    for ktp in range(ST // 2):
                        nc.tensor.matmul(
                            oA, lhsT=v8_sb[:, ktp, :, hA, 0:VW],
                            rhs=pc[:, ktp, :, 0:512],
                            start=(ktp == 0), stop=(ktp == ST // 2 - 1),
                            perf_mode=DR)
                        nc.tensor.matmul(
                            oB, lhsT=v8_sb[:, ktp, :, hB, 0:VW],
                            rhs=pc[:, ktp, :, 512:1024],
                            start=(ktp == 0), stop=(ktp == ST // 2 - 1),
                            perf_mode=DR)
                elif pv8:
                    for ktp in range(ST // 2):
                        p2 = work.tile([128, 2, 1024], fp8, tag="p", bufs=4,
                                       name="p2")
                        for j in range(2):
                            kt = 2 * ktp + j
                            if extra is not None:
                                extra(kt)
                            ks = slice(kt * 128, (kt + 1) * 128)
                            sAB = ps_pool.tile([128, 1024], f32, tag="ps",
                                               name="sAB")
                            nc.tensor.matmul(
                                sAB[:, 0:512], lhsT=kT_sb[0:64, hp, ks],
                                rhs=qT_sb[0:64, hp, qs],
                                start=True, stop=True, tile_position=(0, 0))
                            nc.tensor.matmul(
                                sAB[:, 512:1024], lhsT=kT_sb[64:128, hp, ks],
                                rhs=qT_sb[64:128, hp, qs],
                                start=True, stop=True, tile_position=(64, 0))
                            nc.scalar.activation(out=p2[:, j, :], in_=sAB,
                                                 func=Exp, scale=SCALE)
                        nc.tensor.matmul(
                            oA, lhsT=v8_sb[:, ktp, :, hA, 0:VW],
                            rhs=p2[:, :, 0:512],
                            start=(ktp == 0), stop=(ktp == ST // 2 - 1),
                            perf_mode=DR)
                        nc.tensor.matmul(
                            oB, lhsT=v8_sb[:, ktp, :, hB, 0:VW],
                            rhs=p2[:, :, 512:1024],
                            start=(ktp == 0), stop=(ktp == ST // 2 - 1),
                            perf_mode=DR)
                else:
                  for kt in range(ST):
                    if extra is not None:
                        extra(kt)
                    ks = slice(kt * 128, (kt + 1) * 128)
                    sAB = ps_pool.tile([128, 1024], f32, tag="ps", name="sAB")
                    sA, sB = sAB[:, 0:512], sAB[:, 512:1024]
                    nc.tensor.matmul(
                        sA, lhsT=kT_sb[0:64, hp, ks], rhs=qT_sb[0:64, hp, qs],
                        start=True, stop=True, tile_position=(0, 0))
                    nc.tensor.matmul(
                        sB, lhsT=kT_sb[64:128, hp, ks], rhs=qT_sb[64:128, hp, qs],
                        start=True, stop=True, tile_position=(64, 0))
                    pAB = work.tile([128, 1024], bf16, tag="p", bufs=8, name="pAB")
                    if "act512" in phs:
                        nc.scalar.activation(out=pAB[:, 0:512], in_=sA,
                                             func=Exp, scale=SCALE)
                        nc.scalar.activation(out=pAB[:, 512:1024], in_=sB,
                                             func=Exp, scale=SCALE)
                    else:
                        nc.scalar.activation(out=pAB, in_=sAB, func=Exp, scale=SCALE)
                    pvn = 256 if "pvhalf" in phs else 512
                    nc.tensor.matmul(
                        oA[:, 0:pvn], lhsT=v_sb[:, kt, hA * VW:(hA + 1) * VW],
                        rhs=pAB[:, 0:pvn],
                        start=(kt == 0), stop=(kt == ST - 1))
                    nc.tensor.matmul(
                        oB[:, 0:pvn], lhsT=v_sb[:, kt, hB * VW:(hB + 1) * VW],
                        rhs=pAB[:, 512:512 + pvn],
                        start=(kt == 0), stop=(kt == ST - 1))
                if "nonorm" in phs:
                    return
                # Normalization, off the PSUM critical path: evacuate oA/oB to
                # SBUF immediately (frees the pv banks for the next chunk),
                # then 1/denominator (approx, 18-bit), broadcast across the 64
                # head dims via a K=1 matmul, and multiply on DVE.
                oab = work.tile([VW, 1024], bf16, tag="oab", bufs=3, name="oab")
                nc.vector.tensor_copy(out=oab[:, 0:512], in_=oA)
                nc.vector.tensor_copy(out=oab[:, 512:1024], in_=oB)
                if "oldnorm" in phs:
                    rA = work.tile([1, 512], f32, tag="recip", bufs=4, name="rA")
                    rB = work.tile([1, 512], f32, tag="recip", bufs=4, name="rB")
                    nc.vector.reciprocal(out=rA, in_=oab[D:VW, 0:512])
                    nc.vector.reciprocal(out=rB, in_=oab[D:VW, 512:1024])
                    bcA = work.tile([64, 512], f32, tag="bcs", bufs=2, name="bcA")
                    bcB = work.tile([64, 512], f32, tag="bcs", bufs=2, name="bcB")
                    nc.gpsimd.partition_broadcast(bcA, rA)
                    nc.gpsimd.partition_broadcast(bcB, rB)
                    nc.vector.tensor_mul(
                        out=oT_sb[0:64, hp, qs], in0=oab[0:D, 0:512], in1=bcA)
                    nc.vector.tensor_mul(
                        out=oT_sb[64:128, hp, qs], in0=oab[0:D, 512:1024], in1=bcB)
                    return
                # the custom recip op only works at base partition 0, so plain-
                # copy the denominator row down to partition 0 first.
                oden = work.tile([1, 1024], f32, tag="oden", bufs=2, name="oden")
                nc.vector.tensor_copy(out=oden, in_=oab[D:VW, :])
                rAB = work.tile([1, 1024], f32, tag="recip", bufs=2, name="rAB")
                nc.vector.reciprocal_approx_fast(out=rAB, in_=oden)
                if "gpbc" in phs:
                    bcg = work.tile([D, 1024],
                                    bf16 if "bch" in phs else f32,
                                    tag="bcg", bufs=2, name="bcg")
                    nc.gpsimd.partition_broadcast(bcg, rAB)
                    nc.vector.tensor_mul(
                        out=oT_sb[0:64, hp, qs], in0=oab[0:D, 0:512],
                        in1=bcg[:, 0:512])
                    nc.vector.tensor_mul(
                        out=oT_sb[64:128, hp, qs], in0=oab[0:D, 512:1024],
                        in1=bcg[:, 512:1024])
                    return
                rABh = work.tile([1, 1024], bf16, tag="reciph", bufs=2,
                                 name="rABh")
                nc.vector.tensor_copy(out=rABh, in_=rAB)
                if bc_own:
                    bc = bc_pool.tile([128, 1024], f32, tag="bc", name="bc")
                else:
                    bc = ps_pool.tile([128, 1024], f32, tag="ps", name="bc")
                nc.tensor.matmul(bc[0:D, 0:512], lhsT=ones64h,
                                 rhs=rABh[:, 0:512], start=True, stop=True)
                nc.tensor.matmul(bc[0:D, 512:1024], lhsT=ones64h,
                                 rhs=rABh[:, 512:1024], start=True, stop=True)
                nc.vector.tensor_mul(
                    out=oT_sb[0:64, hp, qs], in0=oab[0:D, 0:512], in1=bc[0:D, 0:512])
                nc.vector.tensor_mul(
                    out=oT_sb[64:128, hp, qs], in0=oab[0:D, 512:1024],
                    in1=bc[0:D, 512:1024])

            def outproj_tile(st):
                acc = acc_pool.tile([128, 1024], f32, tag=acc_tag,
                                    name="out_ps")
                for it in range(IT):
                    for h2 in range(2):
                        nc.tensor.matmul(
                            acc[:, h2 * 512:(h2 + 1) * 512],
                            lhsT=oT_sb[:, it, st * 128:(st + 1) * 128],
                            rhs=wo_sb[:, it, h2 * 512:(h2 + 1) * 512],
                            start=(it == 0), stop=(it == IT - 1))
                ob = outp.tile([128, 1024], bf16, tag="ob", bufs=3, name="ob")
                nc.vector.tensor_copy(out=ob, in_=acc)
                (nc.gpsimd if dmaq else nc.sync).dma_start(
                    out=out_d[st * 128:(st + 1) * 128, :], in_=ob)

            if "attn" in phs:
                # pair-major chunk order; qk projections for the next pair and
                # v tiles are woven into chunk kt-steps so PE slack under the
                # ACT-bound exp stream absorbs them.
                if "proj" in phs:
                    if "noweave" in phs:
                        if preproj:
                            enter_loop()
                        for it in range(IT):
                            proj_qk(it)
                        for st in range(ST):
                            proj_v(st)
                    elif preproj:
                        # first-iteration qk(0) runs once, outside the loop;
                        # steady-state iterations get it from the pair-3 weave.
                        proj_qk(0)
                        enter_loop()
                        input_dmas()
                        for st in range(4):
                            proj_v(st)
                    else:
                        proj_qk(0)
                        for st in range(4):
                            proj_v(st)

                def make_extra(hp, nq):
                    if "proj" not in phs or "noweave" in phs:
                        return None
                    def extra(kt):
                        if hp == 0 and nq == 0 and 4 + kt < ST:
                            proj_v(4 + kt)
                        wrap = preproj and hp == IT - 1
                        if (hp < IT - 1 or wrap) and nq in (1, 2) and kt in (3, 11):
                            acc_idx = (nq - 1) * 2 + (0 if kt == 3 else 1)
                            proj_qk_acc((hp + 1) % IT, acc_idx // 2, acc_idx % 2)
                    return extra

                for hp in range(IT):
                    for nq in range(NQ):
                        attn_chunk(hp, nq, make_extra(hp, nq))
                        if (hp == IT - 1 and "outproj" in phs
                                and "outlate" not in phs):
                            for st in range(4 * nq, 4 * (nq + 1)):
                                outproj_tile(st)
                if "outproj" in phs and "outlate" in phs:
                    for st in range(ST):
                        outproj_tile(st)
            else:
                if preproj:
                    enter_loop()
                if "proj" in phs:
                    for it in range(IT):
                        proj_qk(it)
                    for st in range(ST):
                        proj_v(st)
                if "outproj" in phs:
                    for st in range(ST):
                        outproj_tile(st)

    nc.compile()
    return nc


@functools.lru_cache(maxsize=8)
def _built(repeat=1, phases="dma,proj,attn,outproj,bcpsum,auxacc,pburst4,dmaq,gpbc", fused_exp=True):
    return _build(repeat, phases, fused_exp)


def _pm(a):
    """[T*128, F] -> partition-major [128, T*F] (bf16)."""
    T = a.shape[0] // 128
    return np.ascontiguousarray(
        a.reshape(T, 128, a.shape[1]).swapaxes(0, 1).reshape(128, -1)
    ).astype(ml_dtypes.bfloat16)


def _in_maps(hidden_states, Wq, Wk, Wv, Wo):
    maps = []
    for c in range(NCORES):
        b, half = divmod(c, 2)
        sl = slice(half * I, (half + 1) * I)
        maps.append({
            "xt": _pm(np.ascontiguousarray(hidden_states[b].T)),
            "wq": _pm(Wq[:, sl]),
            "wk": _pm(Wk[:, sl]),
            "wv": _pm(Wv[:, sl]),
            "wo": _pm(Wo[sl, :]),
        })
    return maps


@functools.lru_cache(maxsize=1)
def _runner():
    """Compile the SPMD program once and return a function
    maps -> list of per-core output dicts."""
    import jax
    from jax.sharding import Mesh, PartitionSpec, NamedSharding
    from jax.experimental.shard_map import shard_map

    import concourse.mybir as mybir
    from concourse.bass2jax import (
        _bass_exec_p, install_neuronx_cc_hook, partition_id_tensor)

    nc = _built()
    install_neuronx_cc_hook()
    partition_name = nc.partition_id_tensor.name if nc.partition_id_tensor else None

    in_names, out_names, out_avals, zero_outs = [], [], [], []
    for alloc in nc.m.functions[0].allocations:
        if not isinstance(alloc, mybir.MemoryLocationSet):
            continue
        name = alloc.memorylocations[0].name
        if alloc.kind == "ExternalInput":
            if name != partition_name:
                in_names.append(name)
        elif alloc.kind == "ExternalOutput":
            out_names.append(name)
            shape = tuple(alloc.tensor_shape)
            dtype = mybir.dt.np(alloc.dtype)
            out_avals.append(jax.core.ShapedArray(shape, dtype))
            zero_outs.append(np.zeros(shape, dtype))
    n_params = len(in_names)
    all_in_names = in_names + out_names
    if partition_name is not None:
        all_in_names = all_in_names + [partition_name]

    def _body(*args):
        operands = list(args)
        if partition_name is not None:
            operands.append(partition_id_tensor())
        return tuple(_bass_exec_p.bind(
            *operands,
            out_avals=tuple(out_avals),
            in_names=tuple(all_in_names),
            out_names=tuple(out_names),
            lowering_input_output_aliases=(),
            sim_require_finite=True,
            sim_require_nnan=True,
            nc=nc,
        ))

    devices = jax.devices()[:NCORES]
    mesh = Mesh(np.asarray(devices), ("core",))
    in_specs = (PartitionSpec("core"),) * (n_params + len(out_names))
    out_specs = (PartitionSpec("core"),) * len(out_names)
    sharded = jax.jit(
        shard_map(_body, mesh=mesh, in_specs=in_specs, out_specs=out_specs,
                  check_rep=False),
        keep_unused=True,
    )
    sharding = NamedSharding(mesh, PartitionSpec("core"))
    dev_zero = [jax.device_put(
        np.zeros((NCORES * z.shape[0], *z.shape[1:]), z.dtype), sharding)
        for z in zero_outs]

    def run(maps):
        concat_in = [np.concatenate([np.asarray(maps[c][n]) for c in range(NCORES)],
                                    axis=0) for n in in_names]
        dev_in = [jax.device_put(a, sharding) for a in concat_in]
        out_arrs = sharded(*dev_in, *dev_zero)
        return [
            {n: np.asarray(out_arrs[i]).reshape(NCORES, *out_avals[i].shape)[c]
             for i, n in enumerate(out_names)}
            for c in range(NCORES)
        ]

    return run


def kernel(hidden_states, Wq, Wk, Wv, Wo, bo):
    maps = _in_maps(np.asarray(hidden_states), np.asarray(Wq), np.asarray(Wk),
                    np.asarray(Wv), np.asarray(Wo))
    results = _runner()(maps)
    B = hidden_states.shape[0]
    out = np.empty((B, S, C), np.float32)
    for b in range(B):
        out[b] = (results[2 * b]["out"].astype(np.float32)
                  + results[2 * b + 1]["out"].astype(np.float32))
    out += np.asarray(bo, np.float32)
    return out


# revision 40
# speedup vs baseline: 1.1668x; 1.0394x over previous
"""Multi-head attention (B=4, S=2048, H=16, D=64, C=1024) on 8 NeuronCores.

Sharding: core c handles batch b=c//2 and head-half half=c%2 (8 heads = 512
inner dims).  Each core computes q/k/v projections for its half of the heads,
full softmax attention over S=2048, and a partial output projection through
its 512 rows of Wo.  Host sums the two partials per batch and adds the bias.

Per-core kernel layout (all matmul operands bf16, PSUM accumulation fp32):
  xt    [C=1024, S=2048]   hidden_states[b].T          (host pre-transposed)
  wq/wk/wv [C, I=512]      per-half weight columns
  wo    [I=512, C=1024]    per-half weight rows
  qT,kT [I, S] stored as 4 SBUF tiles [128, 2048]  (head pair per tile)
  v_pad [S, 8*65]          v with a ones column per head (row-sum via matmul)
  scores^T per (pair, qi-chunk, kj-tile): [kj=128, qi=512] via row-tiled
  (K=64) matmul pairs; exp on ScalarE; p@[v|1] accumulated in PSUM over kj.

v2 schedule (default flags "bcpsum,auxacc,pburst4,dmaq"):
  - constant memsets (v ones-columns) hoisted out of the repeat loop
    (a [128,8320] memset measures ~160us on HW);
  - one DMA per input tensor, all on the SP queue; output stores on the
    GPSIMD SWDGE queue so no dma_start wait can stall the ACT queue;
  - quarter-chunk p@v bursts: the scores->exp stream runs 4 kt ahead into
    an SBUF ring, then 8 pv matmuls burst, decoupling the in-order PE
    from the per-kt PE->ACT->PE round trip;
  - softmax normalization off the PSUM critical path: DVE evacuates the
    pv accumulators to SBUF immediately (banks recycle), then
    approx-reciprocal of the ones-column row-sum, GPSIMD
    partition_broadcast into SBUF (the idle engine; a PE K=1 broadcast
    matmul through PSUM measured 40us slower), and a DVE multiply;
  - scores PSUM ring is a pure 2-slot double buffer; proj/outproj
    accumulators get the third bank pair (sharing rings across
    consumer engines convoys the whole pipeline).
Measured ~445us per iteration vs 606us for the v1 baseline; relative
error 2.60e-3 on every compile draw.
"""

import functools

import numpy as np
import ml_dtypes

S = 2048          # sequence length
C = 1024          # query dim
I = 512           # inner dims per core (8 heads x 64)
HC = 8            # heads per core
D = 64            # head dim
NCORES = 8
SCALE = D ** -0.5
CT = C // 128     # 8 c-tiles
IT = I // 128     # 4 i-tiles (head pairs)
ST = S // 128     # 16 s-tiles
NQ = S // 512     # 4 qi chunks
VW = D + 1        # 65: v plus ones column


def _build(repeat=1, phases="dma,proj,attn,outproj,bcpsum,auxacc,pburst4,dmaq,gpbc", fused_exp=True):
    import contextlib

    import concourse.bacc as bacc
    import concourse.tile as tile
    from concourse import mybir

    f32 = mybir.dt.float32
    bf16 = mybir.dt.bfloat16
    fp8 = mybir.dt.float8e4
    DR = mybir.MatmulPerfMode.DoubleRow
    Exp = mybir.ActivationFunctionType.Exp

    nc = bacc.Bacc("TRN2", target_bir_lowering=False, debug=False,
                   num_devices=NCORES)

    # All inputs are host-pre-shuffled to partition-major [128, ...] layouts so
    # every DMA reads long contiguous runs per partition.
    xt_d = nc.dram_tensor("xt", [128, CT * S], bf16, kind="ExternalInput").ap()
    wq_d = nc.dram_tensor("wq", [128, CT * I], bf16, kind="ExternalInput").ap()
    wk_d = nc.dram_tensor("wk", [128, CT * I], bf16, kind="ExternalInput").ap()
    wv_d = nc.dram_tensor("wv", [128, CT * I], bf16, kind="ExternalInput").ap()
    wo_d = nc.dram_tensor("wo", [128, IT * C], bf16, kind="ExternalInput").ap()
    out_d = nc.dram_tensor("out", [S, C], bf16, kind="ExternalOutput").ap()

    with tile.TileContext(nc) as tc:
        with contextlib.ExitStack() as ctx:
            pflags = set(phases.split(","))
            ps3 = "ps3" in pflags and "gpbc" in pflags
            bc_own = "bcpsum" in pflags and not ps3
            aux_acc = "auxacc" in pflags
            const = ctx.enter_context(tc.tile_pool(name="const", bufs=1))
            work = ctx.enter_context(tc.tile_pool(name="work", bufs=4))
            outp = ctx.enter_context(tc.tile_pool(name="outp", bufs=4))
            ps_pool = ctx.enter_context(
                tc.tile_pool(name="ps", bufs=(2 if bc_own else 3), space="PSUM"))
            # with gpbc+ps3 the broadcast lives in SBUF: scores get 3 slots
            # and the proj/outproj accumulators ride the ring's spare slot.
            pv_pool = ctx.enter_context(tc.tile_pool(name="pv", bufs=2, space="PSUM"))
            bc_pool = ctx.enter_context(
                tc.tile_pool(name="bcp", bufs=1, space="PSUM")) if bc_own else None
            # proj/outproj accumulators share the bc banks (used only at chunk
            # ends) so the scores ring stays a pure double-buffer.
            acc_pool = bc_pool if (bc_own and aux_acc) else ps_pool
            acc_tag = "bc" if (bc_own and aux_acc) else "ps"

            # ---- persistent tiles -------------------------------------------
            xt_sb = const.tile([128, CT, S], bf16)
            wq_sb = const.tile([128, CT, I], bf16)
            wk_sb = const.tile([128, CT, I], bf16)
            wv_sb = const.tile([128, CT, I], bf16)
            wo_sb = const.tile([128, IT, C], bf16)
            qT_sb = const.tile([128, IT, S], bf16)
            kT_sb = const.tile([128, IT, S], bf16)
            pv8 = "pv8" in pflags
            if not pv8:
                v_sb = const.tile([128, ST, HC * VW], bf16)
            VW8 = 80
            if pv8:
                v8_sb = const.tile([128, ST // 2, 2, HC, VW8], fp8)
            oT_sb = const.tile([128, IT, S], bf16)
            ones64 = const.tile([1, D], f32)
            ones64h = const.tile([1, D], bf16)

            # ---- one-time setup (outside the repeat loop) -------------------
            # ones columns per head (softmax denominator): contiguous memset of
            # the whole tile — the v copies then overwrite the 64 data columns.
            # This memset measures ~160us on HW, so it must stay out of the
            # steady-state loop.
            if pv8:
                nc.vector.memset(v8_sb, 1.0)
            else:
                nc.vector.memset(v_sb, 1.0)
            nc.vector.memset(ones64, 1.0)
            nc.vector.memset(ones64h, 1.0)

            preproj = "preproj" in pflags
            loop_entered = []

            def enter_loop():
                if repeat > 1:
                    ctx.enter_context(tc.For_i(0, repeat, 1))
                loop_entered.append(True)

            if not preproj:
                enter_loop()

            # ---- input DMAs (one per tensor, contiguous per partition) ------
            # All on the sync (SP) queue: a dma_start's wait blocks its whole
            # issuing queue, and the ACT queue must never stall (exp stream).
            dmaq = "dmaq" in pflags
            wq_eng = nc.sync if dmaq else nc.scalar

            def input_dmas(with_wo=True):
                nc.sync.dma_start(out=xt_sb,
                                  in_=xt_d.rearrange("p (t s) -> p t s", s=S))
                wq_eng.dma_start(out=wq_sb,
                                 in_=wq_d.rearrange("p (t i) -> p t i", i=I))
                wq_eng.dma_start(out=wk_sb,
                                 in_=wk_d.rearrange("p (t i) -> p t i", i=I))
                wq_eng.dma_start(out=wv_sb,
                                 in_=wv_d.rearrange("p (t i) -> p t i", i=I))
                if with_wo:
                    wq_eng.dma_start(out=wo_sb,
                                     in_=wo_d.rearrange("p (t c) -> p t c", c=C))

            input_dmas()

            phs = set(phases.split(","))

            # ---- projections -------------------------------------------------
            if "proj" in phs:
                def proj_qk_acc(it, which, nq):
                    w_sb, o_sb = ((wq_sb, qT_sb), (wk_sb, kT_sb))[which]
                    acc = acc_pool.tile([128, 1024], f32, tag=acc_tag,
                                        name="proj_ps")
                    for ct in range(CT):
                        for h2 in range(2):
                            nc.tensor.matmul(
                                acc[:, h2 * 512:(h2 + 1) * 512],
                                lhsT=w_sb[:, ct, it * 128:(it + 1) * 128],
                                rhs=xt_sb[:, ct, nq * 1024 + h2 * 512:
                                          nq * 1024 + (h2 + 1) * 512],
                                start=(ct == 0), stop=(ct == CT - 1))
                    nc.vector.tensor_copy(
                        out=o_sb[:, it, nq * 1024:(nq + 1) * 1024], in_=acc)

                def proj_qk(it):
                    for which in range(2):
                        for nq in range(S // 1024):
                            proj_qk_acc(it, which, nq)

                v_main = (None if pv8 else v_sb.rearrange(
                    "p t (h e) -> p t h e", e=VW)[:, :, :, 0:D])

                def proj_v(st):
                    acc = acc_pool.tile([128, 1024], f32, tag=acc_tag,
                                        name="v_ps")[:, 0:512]
                    for ct in range(CT):
                        nc.tensor.matmul(
                            acc,
                            lhsT=xt_sb[:, ct, st * 128:(st + 1) * 128],
                            rhs=wv_sb[:, ct, :],
                            start=(ct == 0), stop=(ct == CT - 1))
                    if pv8:
                        nc.vector.tensor_copy(
                            out=v8_sb[:, st // 2, st % 2, :, 0:D],
                            in_=acc.rearrange("p (h d) -> p h d", d=D))
                    else:
                        nc.vector.tensor_copy(
                            out=v_main[:, st],
                            in_=acc.rearrange("p (h d) -> p h d", d=D))

            # ---- attention + interleaved output projection ------------------
            def attn_chunk(hp, nq, extra=None):
                hA, hB = 2 * hp, 2 * hp + 1
                qs = slice(nq * 512, (nq + 1) * 512)
                oA = pv_pool.tile([VW, 512], f32, tag="pv", name="oA")
                oB = pv_pool.tile([VW, 512], f32, tag="pv", name="oB")
                if "pburst4" in phs:
                    # bf16 quarter-chunk bursts: scores+exp stream ahead, pv
                    # runs as 8-matmul bursts every 4 kt — decouples the
                    # in-order PE from the per-kt ACT round trip without fp8.
                    # With "skew", quarter q's burst is traced AFTER quarter
                    # q+1's scores so the in-order PE never sits waiting for
                    # the exp of the quarter it just scored.
                    skew = "skew" in phs
                    phq = [None] * 4

                    def scores_quarter(quarter):
                        ph = work.tile([128, 4, 1024], bf16, tag="p",
                                       bufs=(3 if "ph3" in phs else 2),
                                       name="ph")
                        phq[quarter] = ph
                        for k4 in range(4):
                            kt = quarter * 4 + k4
                            if extra is not None:
                                extra(kt)
                            ks = slice(kt * 128, (kt + 1) * 128)
                            sAB = ps_pool.tile([128, 1024], f32, tag="ps",
                                               name="sAB")
                            nc.tensor.matmul(
                                sAB[:, 0:512], lhsT=kT_sb[0:64, hp, ks],
                                rhs=qT_sb[0:64, hp, qs],
                                start=True, stop=True, tile_position=(0, 0))
                            nc.tensor.matmul(
                                sAB[:, 512:1024], lhsT=kT_sb[64:128, hp, ks],
                                rhs=qT_sb[64:128, hp, qs],
                                start=True, stop=True, tile_position=(64, 0))
                            nc.scalar.activation(out=ph[:, k4, :], in_=sAB,
                                                 func=Exp, scale=SCALE)

                    def burst_quarter(quarter):
                        ph = phq[quarter]
                        for k4 in range(4):
                            kt = quarter * 4 + k4
                            nc.tensor.matmul(
                                oA, lhsT=v_sb[:, kt, hA * VW:(hA + 1) * VW],
                                rhs=ph[:, k4, 0:512],
                                start=(kt == 0), stop=(kt == ST - 1))
                            nc.tensor.matmul(
                                oB, lhsT=v_sb[:, kt, hB * VW:(hB + 1) * VW],
                                rhs=ph[:, k4, 512:1024],
                                start=(kt == 0), stop=(kt == ST - 1))

                    if skew:
                        for quarter in range(5):
                            if quarter < 4:
                                scores_quarter(quarter)
                            if quarter >= 1:
                                burst_quarter(quarter - 1)
                    else:
                        for quarter in range(4):
                            scores_quarter(quarter)
                            burst_quarter(quarter)
                elif pv8 and "pburst" in phs:
                    # whole-chunk p buffer: scores+exp stream fully decoupled
                    # from the pv burst, which pipelines against the NEXT
                    # chunk's exp stream at chunk granularity.
                    pc = work.tile([128, ST // 2, 2, 1024], fp8, tag="pc",
                                   bufs=2, name="pc")
                    for kt in range(ST):
                        if extra is not None:
                            extra(kt)
                        ks = slice(kt * 128, (kt + 1) * 128)
                        sAB = ps_pool.tile([128, 1024], f32, tag="ps",
                                           name="sAB")
                        nc.tensor.matmul(
                            sAB[:, 0:512], lhsT=kT_sb[0:64, hp, ks],
                            rhs=qT_sb[0:64, hp, qs],
                            start=True, stop=True, tile_position=(0, 0))
                        nc.tensor.matmul(
                            sAB[:, 512:1024], lhsT=kT_sb[64:128, hp, ks],
                            rhs=qT_sb[64:128, hp, qs],
                            start=True, stop=True, tile_position=(64, 0))
                        nc.scalar.activation(out=pc[:, kt // 2, kt % 2, :],
                                             in_=sAB, func=Exp, scale=SCALE)
                    for ktp in range(ST // 2):
                        nc.tensor.matmul(
                            oA, lhsT=v8_sb[:, ktp, :, hA, 0:VW],
                            rhs=pc[:, ktp, :, 0:512],
                            start=(ktp == 0), stop=(ktp == ST // 2 - 1),
                            perf_mode=DR)
                        nc.tensor.matmul(
                            oB, lhsT=v8_sb[:, ktp, :, hB, 0:VW],
                            rhs=pc[:, ktp, :, 512:1024],
                            start=(ktp == 0), stop=(ktp == ST // 2 - 1),
                            perf_mode=DR)
                elif pv8:
                    for ktp in range(ST // 2):
                        p2 = work.tile([128, 2, 1024], fp8, tag="p", bufs=4,
                                       name="p2")
                        for j in range(2):
                            kt = 2 * ktp + j
                            if extra is not None:
                                extra(kt)
                            ks = slice(kt * 128, (kt + 1) * 128)
                            sAB = ps_pool.tile([128, 1024], f32, tag="ps",
                                               name="sAB")
                            nc.tensor.matmul(
                                sAB[:, 0:512], lhsT=kT_sb[0:64, hp, ks],
                                rhs=qT_sb[0:64, hp, qs],
                                start=True, stop=True, tile_position=(0, 0))
                            nc.tensor.matmul(
                                sAB[:, 512:1024], lhsT=kT_sb[64:128, hp, ks],
                                rhs=qT_sb[64:128, hp, qs],
                                start=True, stop=True, tile_position=(64, 0))
                            nc.scalar.activation(out=p2[:, j, :], in_=sAB,
                                                 func=Exp, scale=SCALE)
                        nc.tensor.matmul(
                            oA, lhsT=v8_sb[:, ktp, :, hA, 0:VW],
                            rhs=p2[:, :, 0:512],
                            start=(ktp == 0), stop=(ktp == ST // 2 - 1),
                            perf_mode=DR)
                        nc.tensor.matmul(
                            oB, lhsT=v8_sb[:, ktp, :, hB, 0:VW],
                            rhs=p2[:, :, 512:1024],
                            start=(ktp == 0), stop=(ktp == ST // 2 - 1),
                            perf_mode=DR)
                else:
                  for kt in range(ST):
                    if extra is not None:
                        extra(kt)
                    ks = slice(kt * 128, (kt + 1) * 128)
                    sAB = ps_pool.tile([128, 1024], f32, tag="ps", name="sAB")
                    sA, sB = sAB[:, 0:512], sAB[:, 512:1024]
                    nc.tensor.matmul(
                        sA, lhsT=kT_sb[0:64, hp, ks], rhs=qT_sb[0:64, hp, qs],
                        start=True, stop=True, tile_position=(0, 0))
                    nc.tensor.matmul(
                        sB, lhsT=kT_sb[64:128, hp, ks], rhs=qT_sb[64:128, hp, qs],
                        start=True, stop=True, tile_position=(64, 0))
                    pAB = work.tile([128, 1024], bf16, tag="p", bufs=8, name="pAB")
                    if "act512" in phs:
                        nc.scalar.activation(out=pAB[:, 0:512], in_=sA,
                                             func=Exp, scale=SCALE)
                        nc.scalar.activation(out=pAB[:, 512:1024], in_=sB,
                                             func=Exp, scale=SCALE)
                    else:
                        nc.scalar.activation(out=pAB, in_=sAB, func=Exp, scale=SCALE)
                    pvn = 256 if "pvhalf" in phs else 512
                    nc.tensor.matmul(
                        oA[:, 0:pvn], lhsT=v_sb[:, kt, hA * VW:(hA + 1) * VW],
                        rhs=pAB[:, 0:pvn],
                        start=(kt == 0), stop=(kt == ST - 1))
                    nc.tensor.matmul(
                        oB[:, 0:pvn], lhsT=v_sb[:, kt, hB * VW:(hB + 1) * VW],
                        rhs=pAB[:, 512:512 + pvn],
                        start=(kt == 0), stop=(kt == ST - 1))
                if "nonorm" in phs:
                    return
                # Normalization, off the PSUM critical path: evacuate oA/oB to
                # SBUF immediately (frees the pv banks for the next chunk),
                # then 1/denominator (approx, 18-bit), broadcast across the 64
                # head dims via a K=1 matmul, and multiply on DVE.
                oab = work.tile([VW, 1024], bf16, tag="oab",
                                bufs=(4 if "rings" in phs else 3), name="oab")
                nc.vector.tensor_copy(out=oab[:, 0:512], in_=oA)
                nc.vector.tensor_copy(out=oab[:, 512:1024], in_=oB)
                if "oldnorm" in phs:
                    rA = work.tile([1, 512], f32, tag="recip", bufs=4, name="rA")
                    rB = work.tile([1, 512], f32, tag="recip", bufs=4, name="rB")
                    nc.vector.reciprocal(out=rA, in_=oab[D:VW, 0:512])
                    nc.vector.reciprocal(out=rB, in_=oab[D:VW, 512:1024])
                    bcA = work.tile([64, 512], f32, tag="bcs", bufs=2, name="bcA")
                    bcB = work.tile([64, 512], f32, tag="bcs", bufs=2, name="bcB")
                    nc.gpsimd.partition_broadcast(bcA, rA)
                    nc.gpsimd.partition_broadcast(bcB, rB)
                    nc.vector.tensor_mul(
                        out=oT_sb[0:64, hp, qs], in0=oab[0:D, 0:512], in1=bcA)
                    nc.vector.tensor_mul(
                        out=oT_sb[64:128, hp, qs], in0=oab[0:D, 512:1024], in1=bcB)
                    return
                # the custom recip op only works at base partition 0, so plain-
                # copy the denominator row down to partition 0 first.
                oden = work.tile([1, 1024], f32, tag="oden", bufs=2, name="oden")
                nc.vector.tensor_copy(out=oden, in_=oab[D:VW, :])
                rAB = work.tile([1, 1024], f32, tag="recip", bufs=2, name="rAB")
                nc.vector.reciprocal_approx_fast(out=rAB, in_=oden)
                if "gpbc" in phs:
                    if "bch" in phs:
                        rABh = work.tile([1, 1024], bf16, tag="reciph",
                                         bufs=2, name="rABh")
                        nc.vector.tensor_copy(out=rABh, in_=rAB)
                        bcg = work.tile([D, 1024], bf16, tag="bcg", bufs=2,
                                        name="bcg")
                        nc.gpsimd.partition_broadcast(bcg, rABh)
                    else:
                        bcg = work.tile([D, 1024], f32, tag="bcg", bufs=2,
                                        name="bcg")
                        nc.gpsimd.partition_broadcast(bcg, rAB)
                    nc.vector.tensor_mul(
                        out=oT_sb[0:64, hp, qs], in0=oab[0:D, 0:512],
                        in1=bcg[:, 0:512])
                    nc.vector.tensor_mul(
                        out=oT_sb[64:128, hp, qs], in0=oab[0:D, 512:1024],
                        in1=bcg[:, 512:1024])
                    return
                rABh = work.tile([1, 1024], bf16, tag="reciph", bufs=2,
                                 name="rABh")
                nc.vector.tensor_copy(out=rABh, in_=rAB)
                if bc_own:
                    bc = bc_pool.tile([128, 1024], f32, tag="bc", name="bc")
                else:
                    bc = ps_pool.tile([128, 1024], f32, tag="ps", name="bc")
                nc.tensor.matmul(bc[0:D, 0:512], lhsT=ones64h,
                                 rhs=rABh[:, 0:512], start=True, stop=True)
                nc.tensor.matmul(bc[0:D, 512:1024], lhsT=ones64h,
                                 rhs=rABh[:, 512:1024], start=True, stop=True)
                nc.vector.tensor_mul(
                    out=oT_sb[0:64, hp, qs], in0=oab[0:D, 0:512], in1=bc[0:D, 0:512])
                nc.vector.tensor_mul(
                    out=oT_sb[64:128, hp, qs], in0=oab[0:D, 512:1024],
                    in1=bc[0:D, 512:1024])

            def outproj_tile(st):
                acc = acc_pool.tile([128, 1024], f32, tag=acc_tag,
                                    name="out_ps")
                for it in range(IT):
                    for h2 in range(2):
                        nc.tensor.matmul(
                            acc[:, h2 * 512:(h2 + 1) * 512],
                            lhsT=oT_sb[:, it, st * 128:(st + 1) * 128],
                            rhs=wo_sb[:, it, h2 * 512:(h2 + 1) * 512],
                            start=(it == 0), stop=(it == IT - 1))
                ob = outp.tile([128, 1024], bf16, tag="ob",
                               bufs=(4 if "rings" in phs else 3), name="ob")
                nc.vector.tensor_copy(out=ob, in_=acc)
                (nc.gpsimd if dmaq else nc.sync).dma_start(
                    out=out_d[st * 128:(st + 1) * 128, :], in_=ob)

            if "attn" in phs:
                # pair-major chunk order; qk projections for the next pair and
                # v tiles are woven into chunk kt-steps so PE slack under the
                # ACT-bound exp stream absorbs them.
                if "proj" in phs:
                    if "noweave" in phs:
                        if preproj:
                            enter_loop()
                        for it in range(IT):
                            proj_qk(it)
                        for st in range(ST):
                            proj_v(st)
                    elif preproj:
                        # first-iteration qk(0) runs once, outside the loop;
                        # steady-state iterations get it from the pair-3 weave.
                        proj_qk(0)
                        enter_loop()
                        input_dmas()
                        for st in range(4):
                            proj_v(st)
                    else:
                        proj_qk(0)
                        for st in range(4):
                            proj_v(st)

                def make_extra(hp, nq):
                    if "proj" not in phs or "noweave" in phs:
                        return None
                    def extra(kt):
                        if hp == 0 and nq == 0 and 4 + kt < ST:
                            proj_v(4 + kt)
                        wrap = preproj and hp == IT - 1
                        if (hp < IT - 1 or wrap) and nq in (1, 2) and kt in (3, 11):
                            acc_idx = (nq - 1) * 2 + (0 if kt == 3 else 1)
                            proj_qk_acc((hp + 1) % IT, acc_idx // 2, acc_idx % 2)
                    return extra

                for hp in range(IT):
                    for nq in range(NQ):
                        attn_chunk(hp, nq, make_extra(hp, nq))
                        if (hp == IT - 1 and "outproj" in phs
                                and "outlate" not in phs):
                            for st in range(4 * nq, 4 * (nq + 1)):
                                outproj_tile(st)
                if "outproj" in phs and "outlate" in phs:
                    for st in range(ST):
                        outproj_tile(st)
            else:
                if preproj:
                    enter_loop()
                if "proj" in phs:
                    for it in range(IT):
                        proj_qk(it)
                    for st in range(ST):
                        proj_v(st)
                if "outproj" in phs:
                    for st in range(ST):
                        outproj_tile(st)

    nc.compile()
    return nc


@functools.lru_cache(maxsize=8)
def _built(repeat=1, phases="dma,proj,attn,outproj,bcpsum,auxacc,pburst4,dmaq,gpbc", fused_exp=True):
    return _build(repeat, phases, fused_exp)


def _pm(a):
    """[T*128, F] -> partition-major [128, T*F] (bf16)."""
    T = a.shape[0] // 128
    return np.ascontiguousarray(
        a.reshape(T, 128, a.shape[1]).swapaxes(0, 1).reshape(128, -1)
    ).astype(ml_dtypes.bfloat16)


def _in_maps(hidden_states, Wq, Wk, Wv, Wo):
    maps = []
    for c in range(NCORES):
        b, half = divmod(c, 2)
        sl = slice(half * I, (half + 1) * I)
        maps.append({
            "xt": _pm(np.ascontiguousarray(hidden_states[b].T)),
            "wq": _pm(Wq[:, sl]),
            "wk": _pm(Wk[:, sl]),
            "wv": _pm(Wv[:, sl]),
            "wo": _pm(Wo[sl, :]),
        })
    return maps


@functools.lru_cache(maxsize=1)
def _runner():
    """Compile the SPMD program once and return a function
    maps -> list of per-core output dicts."""
    import jax
    from jax.sharding import Mesh, PartitionSpec, NamedSharding
    from jax.experimental.shard_map import shard_map

    import concourse.mybir as mybir
    from concourse.bass2jax import (
        _bass_exec_p, install_neuronx_cc_hook, partition_id_tensor)

    nc = _built()
    install_neuronx_cc_hook()
    partition_name = nc.partition_id_tensor.name if nc.partition_id_tensor else None

    in_names, out_names, out_avals, zero_outs = [], [], [], []
    for alloc in nc.m.functions[0].allocations:
        if not isinstance(alloc, mybir.MemoryLocationSet):
            continue
        name = alloc.memorylocations[0].name
        if alloc.kind == "ExternalInput":
            if name != partition_name:
                in_names.append(name)
        elif alloc.kind == "ExternalOutput":
            out_names.append(name)
            shape = tuple(alloc.tensor_shape)
            dtype = mybir.dt.np(alloc.dtype)
            out_avals.append(jax.core.ShapedArray(shape, dtype))
            zero_outs.append(np.zeros(shape, dtype))
    n_params = len(in_names)
    all_in_names = in_names + out_names
    if partition_name is not None:
        all_in_names = all_in_names + [partition_name]

    def _body(*args):
        operands = list(args)
        if partition_name is not None:
            operands.append(partition_id_tensor())
        return tuple(_bass_exec_p.bind(
            *operands,
            out_avals=tuple(out_avals),
            in_names=tuple(all_in_names),
            out_names=tuple(out_names),
            lowering_input_output_aliases=(),
            sim_require_finite=True,
            sim_require_nnan=True,
            nc=nc,
        ))

    devices = jax.devices()[:NCORES]
    mesh = Mesh(np.asarray(devices), ("core",))
    in_specs = (PartitionSpec("core"),) * (n_params + len(out_names))
    out_specs = (PartitionSpec("core"),) * len(out_names)
    sharded = jax.jit(
        shard_map(_body, mesh=mesh, in_specs=in_specs, out_specs=out_specs,
                  check_rep=False),
        keep_unused=True,
    )
    sharding = NamedSharding(mesh, PartitionSpec("core"))
    dev_zero = [jax.device_put(
        np.zeros((NCORES * z.shape[0], *z.shape[1:]), z.dtype), sharding)
        for z in zero_outs]

    def run(maps):
        concat_in = [np.concatenate([np.asarray(maps[c][n]) for c in range(NCORES)],
                                    axis=0) for n in in_names]
        dev_in = [jax.device_put(a, sharding) for a in concat_in]
        out_arrs = sharded(*dev_in, *dev_zero)
        return [
            {n: np.asarray(out_arrs[i]).reshape(NCORES, *out_avals[i].shape)[c]
             for i, n in enumerate(out_names)}
            for c in range(NCORES)
        ]

    return run


def kernel(hidden_states, Wq, Wk, Wv, Wo, bo):
    maps = _in_maps(np.asarray(hidden_states), np.asarray(Wq), np.asarray(Wk),
                    np.asarray(Wv), np.asarray(Wo))
    results = _runner()(maps)
    B = hidden_states.shape[0]
    out = np.empty((B, S, C), np.float32)
    for b in range(B):
        out[b] = (results[2 * b]["out"].astype(np.float32)
                  + results[2 * b + 1]["out"].astype(np.float32))
    out += np.asarray(bo, np.float32)
    return out
